# revision 9
# baseline (speedup 1.0000x reference)
"""Trainium2 Bass kernel for nn_BertIntermediate (QuantizeLinear + exact GELU).

Reference computation (see harness reference):
    xq = fake_quant(x)   # symmetric per-tensor int8 fake quant, scale = max|x|/127
    Wq = fake_quant(W)
    h  = xq @ Wq.T + b
    out = h * 0.5 * (1 + erf(h/sqrt(2)))

Key numerical insight: q = round(v/scale) is an integer in [-127, 127], exactly
representable in bf16 (8-bit mantissa holds integers up to 256). Products are
<= 127*128 and k-sums over H=1024 stay below 2^24, so a bf16 matmul with fp32
PSUM accumulation reproduces the fp32 reference EXACTLY (up to rounding-tie
flips worth ~1e-3 absolute). The scales factor out of the GEMM:
    h = (sx*sw) * (qx @ qW.T) + b
and fold into the scalar-engine activation (gelu(scale*psum + bias)).

Sharding (8 cores): 2D grid, 4-way over tokens x 2-way over intermediate dim.
Per core: x^T quarter [1024, 2048] replicated x2, W^T half [1024, 2048]
replicated x4, output block written transposed [2048 I, 2048 tok].
This costs ~34 MB DMA per core vs ~53 MB for the pure Megatron column split,
balancing DMA (~100us) against PE (~110us) at the roofline ridge.

The global quantization scales need max|x|, max|W| over the FULL tensors: each
core reduces a distinct 1/8 shard (passed as extra inputs), then a tiny
AllReduce(max) of 2 floats across the 8 cores combines them on-device.

Rounding: round-half-to-even via the fp32 magic-number trick
    rne(v) = (v + 1.5*2^23) - 1.5*2^23        (exact for |v| <= 2^22)
which matches jnp.round's banker's rounding.
"""

import numpy as np

import concourse.bass as bass
import concourse.bass_isa as bass_isa
import concourse.mybir as mybir
from concourse import bass_utils
from concourse.tile import TileContext

F32 = mybir.dt.float32
BF16 = mybir.dt.bfloat16
MAGIC = 12582912.0  # 1.5 * 2**23: fp32 add/sub rounds to nearest int (RNE)
N_CORES = 8
TI, II = 4, 2  # token-quarters x intermediate-halves

# Full problem dims
B, S, H, I = 16, 512, 1024, 4096
M = B * S  # 8192 tokens


def _split_sync_waits(nc, max_waits=1):
    """Walrus in this container rejects instructions carrying more than a
    couple of sync-wait commands ("Too many sync wait commands"). Hoist excess
    waits onto single-wait nops inserted just before the instruction on the
    same engine queue — sequencers process in order, so semantics are
    unchanged."""
    n = 0
    for fn in nc.m.functions:
        for blk in fn.blocks:
            new_insts = []
            for inst in blk.instructions:
                si = inst.sync_info
                waits = list(si.on_wait or []) if si is not None else []
                if len(waits) > max_waits:
                    keep = waits[-max_waits:]
                    for w in waits[:-max_waits]:
                        n += 1
                        nop = mybir.InstNoOp(
                            name=f"I-waitsplit-{n}",
                            ins=[],
                            outs=[],
                            engine=inst.engine,
                        )
                        nop.sync_info = mybir.SyncInfo(on_wait=[w], on_update=[])
                        new_insts.append(nop)
                    inst.sync_info = mybir.SyncInfo(
                        on_wait=keep, on_update=list(si.on_update or [])
                    )
                new_insts.append(inst)
            blk.instructions = new_insts


def build(h=H, m_core=M // TI, i_core=I // II, xsh_cols=None, wsh_cols=None):
    """Build the SPMD Bass program for one core's block.

    h:      contraction dim (multiple of 128)
    m_core: tokens per core (multiple of 512)
    i_core: intermediate outputs per core (multiple of 128)
    """
    if xsh_cols is None:
        xsh_cols = m_core // II
    if wsh_cols is None:
        wsh_cols = i_core // TI
    kt = h // 128          # contraction tiles
    n_it = i_core // 128   # output I-tiles (PSUM partition dim)
    n_tg = m_core // 512   # token groups (PSUM free dim)
    n_bt = (n_it + 15) // 16  # bias tile columns padded into [128, n_it]

    nc = bass.Bass(num_devices=N_CORES)
    xT = nc.dram_tensor("xT", [h, m_core], F32, kind="ExternalInput")
    wT = nc.dram_tensor("wT", [h, i_core], F32, kind="ExternalInput")
    wsh = nc.dram_tensor("wsh", [h, wsh_cols], F32, kind="ExternalInput")
    bias = nc.dram_tensor("bias", [128, n_it], F32, kind="ExternalInput")
    outT = nc.dram_tensor("outT", [i_core, m_core], F32, kind="ExternalOutput")
    cc_in = nc.dram_tensor("cc_in", [1, 2], F32, kind="Internal")
    cc_out = nc.dram_tensor("cc_out", [1, 2], F32, kind="Internal", addr_space="Shared")
    scr = nc.dram_tensor("scr", [128, 2], F32, kind="Internal")

    groups = [list(range(N_CORES))]

    with TileContext(nc) as tc:
        with (
            tc.tile_pool(name="res", bufs=1) as res,
            tc.tile_pool(name="stage", bufs=3) as stage,
            tc.tile_pool(name="small", bufs=1) as small,
            tc.tile_pool(name="psum", bufs=8, space="PSUM") as pp,
            tc.tile_pool(name="evac", bufs=4) as evac,
        ):
            # ---- phase 0: stage x fully; local maxes; AllReduce; scales ----
            # x chunks stage in SBUF during the reduction/collective so the
            # quantize pass after the scales arrive does no DMA. Core pairs
            # reduce the same token quarter (duplicated coverage is fine for a
            # max); W gets a distinct per-core 1/8 shard input instead since
            # its half streams only later during the matmul phase.
            xstage = [
                stage.tile([128, m_core], F32, tag="xstage", bufs=kt,
                           name=f"xstage{k}")
                for k in range(kt)
            ]
            macc = small.tile([128, 2 * kt], F32, tag="macc")
            for k in range(kt):
                nc.gpsimd.dma_start(xstage[k][:], xT[k * 128:(k + 1) * 128, :])
                nc.vector.tensor_reduce(
                    macc[:, k:k + 1], xstage[k][:], axis=mybir.AxisListType.X,
                    op=mybir.AluOpType.max, apply_absolute_value=True,
                )
            for k in range(kt):
                ws = stage.tile([128, wsh_cols], F32, tag="ws")
                nc.gpsimd.dma_start(ws[:], wsh[k * 128:(k + 1) * 128, :])
                nc.vector.tensor_reduce(
                    macc[:, kt + k:kt + k + 1], ws[:], axis=mybir.AxisListType.X,
                    op=mybir.AluOpType.max, apply_absolute_value=True,
                )
            gm2 = small.tile([128, 2], F32, tag="gm2")
            nc.vector.tensor_reduce(
                gm2[:, 0:1], macc[:, 0:kt], axis=mybir.AxisListType.X,
                op=mybir.AluOpType.max,
            )
            nc.vector.tensor_reduce(
                gm2[:, 1:2], macc[:, kt:2 * kt], axis=mybir.AxisListType.X,
                op=mybir.AluOpType.max,
            )
            # partition-max via DRAM round-trip transposing gather (the custom
            # GPSIMD partition ops fail codegen in this walrus build)
            nc.sync.dma_start(scr[:, :], gm2[:])
            g3 = small.tile([1, 2, 128], F32, tag="g3")
            nc.sync.dma_start(g3[:], bass.AP(scr, 0, [[0, 1], [1, 2], [2, 128]]))
            lmax = small.tile([1, 2], F32, tag="lmax")
            nc.vector.tensor_reduce(
                lmax[:], g3[:], axis=mybir.AxisListType.X, op=mybir.AluOpType.max
            )
            # AllReduce(max) of [max|x|, max|W|] across the 8 cores
            nc.sync.dma_start(cc_in[:, :], lmax[:])
            nc.gpsimd.collective_compute(
                "AllReduce", mybir.AluOpType.max, replica_groups=groups,
                ins=[cc_in[:, :]], outs=[cc_out[:, :]],
            )
            gmx = small.tile([128, 2], F32, tag="gmx")
            nc.sync.dma_start(gmx[:], cc_out[0:1, :].broadcast_to([128, 2]))
            # scales: s = gmax/127 (jnp computes max/127; mult by 1/127 is
            # within 1 ulp), inv = 1/s, ss = sx*sw
            sxsw = small.tile([128, 2], F32, tag="sxsw")
            nc.vector.tensor_scalar_mul(sxsw[:], gmx[:], 1.0 / 127.0)
            inv = small.tile([128, 2], F32, tag="inv")
            nc.vector.reciprocal(inv[:], sxsw[:])
            ss = small.tile([128, 1], F32, tag="ss")
            nc.vector.tensor_tensor(
                ss[:], sxsw[:, 0:1], sxsw[:, 1:2], op=mybir.AluOpType.mult
            )
            bt = small.tile([128, n_it], F32, tag="bt")
            nc.sync.dma_start(bt[:], bias[:, :])

            # ---- phase 1: quantize x from the staged chunks (no DMA) ----
            CH = 1024  # ACT/DVE sub-chunk width
            xq = res.tile([128, kt * m_core], BF16, tag="xq")  # [p, k, tok]
            wq = res.tile([128, kt * i_core], BF16, tag="wq")  # [p, k, I]
            for k in range(kt):
                for c0 in range(0, m_core, CH):
                    cw = min(CH, m_core - c0)
                    t1 = stage.tile([128, CH], F32, tag="t1")
                    nc.scalar.activation(
                        t1[:, :cw], xstage[k][:, c0:c0 + cw],
                        mybir.ActivationFunctionType.Copy,
                        bias=MAGIC, scale=inv[:, 0:1],
                    )
                    nc.vector.tensor_scalar(
                        xq[:, k * m_core + c0:k * m_core + c0 + cw],
                        t1[:, :cw], MAGIC, None, op0=mybir.AluOpType.subtract,
                    )

            # ---- phase 2: stream+quantize W by I-quads; matmul + gelu ----
            IQ = 512  # W I-columns per streamed chunk = 4 output I-tiles
            for q0 in range(0, i_core, IQ):
                qw = min(IQ, i_core - q0)
                for k in range(kt):
                    wf = stage.tile([128, IQ], F32, tag="wf")
                    nc.sync.dma_start(
                        wf[:, :qw], wT[k * 128:(k + 1) * 128, q0:q0 + qw]
                    )
                    t2 = stage.tile([128, IQ], F32, tag="t2")
                    nc.scalar.activation(
                        t2[:, :qw], wf[:, :qw],
                        mybir.ActivationFunctionType.Copy,
                        bias=MAGIC, scale=inv[:, 1:2],
                    )
                    nc.vector.tensor_scalar(
                        wq[:, k * i_core + q0:k * i_core + q0 + qw],
                        t2[:, :qw], MAGIC, None, op0=mybir.AluOpType.subtract,
                    )
                for i in range(q0 // 128, (q0 + qw) // 128):
                    ps = [
                        pp.tile([128, 512], F32, tag="ps", name=f"ps_{i}_{tg}")
                        for tg in range(n_tg)
                    ]
                    for k in range(kt):
                        lhsT = wq[:, k * i_core + i * 128:k * i_core + (i + 1) * 128]
                        for tg in range(n_tg):
                            rhs = xq[:, k * m_core + tg * 512:k * m_core + (tg + 1) * 512]
                            nc.tensor.matmul(
                                ps[tg][:], lhsT, rhs,
                                start=(k == 0), stop=(k == kt - 1),
                            )
                    for tg in range(n_tg):
                        ot = evac.tile([128, 512], F32, tag="ot")
                        nc.scalar.activation(
                            ot[:], ps[tg][:],
                            mybir.ActivationFunctionType.Gelu,
                            bias=bt[:, i:i + 1], scale=ss[:, 0:1],
                        )
                        nc.sync.dma_start(
                            outT[i * 128:(i + 1) * 128, tg * 512:(tg + 1) * 512],
                            ot[:],
                        )
    _split_sync_waits(nc)
    return nc


_CACHE: dict = {}


def _get_nc():
    if "nc" not in _CACHE:
        _CACHE["nc"] = build()
    return _CACHE["nc"]


def shard_inputs(x, W, b):
    """Host-side sharding: pure layout (transpose/slice/replicate), no math."""
    x2 = np.ascontiguousarray(x.reshape(M, H).T)  # [H, M]
    in_maps = []
    for c in range(N_CORES):
        ti, ii = c // II, c % II
        mq, ih = M // TI, I // II
        xT = np.ascontiguousarray(x2[:, ti * mq:(ti + 1) * mq])
        wT = np.ascontiguousarray(W[ii * ih:(ii + 1) * ih, :].T)
        # distinct 1/8 shard of W for the global max reduction (x maxes come
        # from the staged full quarter on-device, duplicated across core pairs)
        wss = np.ascontiguousarray(wT[:, ti * (ih // TI):(ti + 1) * (ih // TI)])
        bia = np.ascontiguousarray(
            b[ii * ih:(ii + 1) * ih].reshape(ih // 128, 128).T
        )
        in_maps.append({"xT": xT, "wT": wT, "wsh": wss, "bias": bia})
    return in_maps


def unshard_output(results):
    """Assemble per-core transposed blocks into the full [B, S, I] output."""
    outT = np.empty((I, M), np.float32)
    for c in range(N_CORES):
        ti, ii = c // II, c % II
        mq, ih = M // TI, I // II
        outT[ii * ih:(ii + 1) * ih, ti * mq:(ti + 1) * mq] = results[c]["outT"]
    return np.ascontiguousarray(outT.T).reshape(B, S, I)


def kernel(x, W, b):
    nc = _get_nc()
    in_maps = shard_inputs(
        np.asarray(x, np.float32), np.asarray(W, np.float32), np.asarray(b, np.float32)
    )
    res = bass_utils.run_bass_kernel_spmd(nc, in_maps, core_ids=list(range(N_CORES)))
    return unshard_output(res.results)


# revision 15
# speedup vs baseline: 1.1645x; 1.1645x over previous
"""Trainium2 Bass kernel for nn_BertIntermediate (QuantizeLinear + exact GELU).

Reference computation (see harness reference):
    xq = fake_quant(x)   # symmetric per-tensor int8 fake quant, scale = max|x|/127
    Wq = fake_quant(W)
    h  = xq @ Wq.T + b
    out = h * 0.5 * (1 + erf(h/sqrt(2)))

Key numerical insight: q = round(v/scale) is an integer in [-127, 127], exactly
representable in bf16 (8-bit mantissa holds integers up to 256). Products are
<= 127*128 and k-sums over H=1024 stay below 2^24, so a bf16 matmul with fp32
PSUM accumulation reproduces the fp32 reference EXACTLY (up to rounding-tie
flips worth ~1e-3 absolute). The scales factor out of the GEMM:
    h = (sx*sw) * (qx @ qW.T) + b
and fold into the scalar-engine activation (gelu(scale*psum + bias)).

Sharding (8 cores): 2D grid, 4-way over tokens x 2-way over intermediate dim.
Per core: x^T quarter [1024, 2048] replicated x2, W^T half [1024, 2048]
replicated x4, output block written transposed [2048 I, 2048 tok].
This costs ~34 MB DMA per core vs ~53 MB for the pure Megatron column split,
balancing DMA (~100us) against PE (~110us) at the roofline ridge.

The global quantization scales need max|x|, max|W| over the FULL tensors: each
core reduces a distinct 1/8 shard (passed as extra inputs), then a tiny
AllReduce(max) of 2 floats across the 8 cores combines them on-device.

Rounding: round-half-to-even via the fp32 magic-number trick
    rne(v) = (v + 1.5*2^23) - 1.5*2^23        (exact for |v| <= 2^22)
which matches jnp.round's banker's rounding.
"""

import numpy as np

import concourse.bass as bass
import concourse.bass_isa as bass_isa
import concourse.mybir as mybir
from concourse import bass_utils
from concourse.tile import TileContext
from concourse.tile_rust import add_dep_helper

F32 = mybir.dt.float32
BF16 = mybir.dt.bfloat16
MAGIC = 12582912.0  # 1.5 * 2**23: fp32 add/sub rounds to nearest int (RNE)
N_CORES = 8
TI, II = 4, 2  # token-quarters x intermediate-halves

# Full problem dims
B, S, H, I = 16, 512, 1024, 4096
M = B * S  # 8192 tokens


def _split_sync_waits(nc, max_waits=1):
    """Walrus in this container rejects instructions carrying more than a
    couple of sync-wait commands ("Too many sync wait commands"). Hoist excess
    waits onto single-wait nops inserted just before the instruction on the
    same engine queue — sequencers process in order, so semantics are
    unchanged."""
    n = 0
    for fn in nc.m.functions:
        for blk in fn.blocks:
            new_insts = []
            for inst in blk.instructions:
                si = inst.sync_info
                waits = list(si.on_wait or []) if si is not None else []
                if len(waits) > max_waits:
                    keep = waits[-max_waits:]
                    for w in waits[:-max_waits]:
                        n += 1
                        nop = mybir.InstNoOp(
                            name=f"I-waitsplit-{n}",
                            ins=[],
                            outs=[],
                            engine=inst.engine,
                        )
                        nop.sync_info = mybir.SyncInfo(on_wait=[w], on_update=[])
                        new_insts.append(nop)
                    inst.sync_info = mybir.SyncInfo(
                        on_wait=keep, on_update=list(si.on_update or [])
                    )
                new_insts.append(inst)
            blk.instructions = new_insts


def build(h=H, m_core=M // TI, i_core=I // II, xsh_cols=None, wsh_cols=None):
    """Build the SPMD Bass program for one core's block.

    h:      contraction dim (multiple of 128)
    m_core: tokens per core (multiple of 512)
    i_core: intermediate outputs per core (multiple of 128)
    """
    if xsh_cols is None:
        xsh_cols = m_core // II
    if wsh_cols is None:
        wsh_cols = i_core // TI
    kt = h // 128          # contraction tiles
    n_it = i_core // 128   # output I-tiles (PSUM partition dim)
    n_tg = m_core // 512   # token groups (PSUM free dim)
    n_bt = (n_it + 15) // 16  # bias tile columns padded into [128, n_it]

    nc = bass.Bass(num_devices=N_CORES)
    xT = nc.dram_tensor("xT", [h, m_core], F32, kind="ExternalInput")
    wT = nc.dram_tensor("wT", [h, i_core], F32, kind="ExternalInput")
    wsh = nc.dram_tensor("wsh", [h, wsh_cols], F32, kind="ExternalInput")
    bias = nc.dram_tensor("bias", [128, n_it], F32, kind="ExternalInput")
    outT = nc.dram_tensor("outT", [i_core, m_core], F32, kind="ExternalOutput")
    cc_in = nc.dram_tensor("cc_in", [1, 2], F32, kind="Internal")
    cc_out = nc.dram_tensor("cc_out", [1, 2], F32, kind="Internal", addr_space="Shared")
    scr = nc.dram_tensor("scr", [128, 2], F32, kind="Internal")

    groups = [list(range(N_CORES))]

    with TileContext(nc) as tc:
        with (
            tc.tile_pool(name="res", bufs=1) as res,
            tc.tile_pool(name="stage", bufs=3) as stage,
            tc.tile_pool(name="small", bufs=1) as small,
            tc.tile_pool(name="psum", bufs=8, space="PSUM") as pp,
            tc.tile_pool(name="evac", bufs=4) as evac,
        ):
            # ---- phase 0: stage x fully; local maxes; AllReduce; scales ----
            # x chunks stage in SBUF during the reduction/collective so the
            # quantize pass after the scales arrive does no DMA. Core pairs
            # reduce the same token quarter (duplicated coverage is fine for a
            # max); W gets a distinct per-core 1/8 shard input instead since
            # its half streams only later during the matmul phase.
            xstage = [
                stage.tile([128, m_core], F32, tag="xstage", bufs=kt,
                           name=f"xstage{k}")
                for k in range(kt)
            ]
            macc = small.tile([128, 2 * kt], F32, tag="macc")
            # The host permutes each core's token columns so its distinct 1/8
            # max-shard sits in columns [0, xsh_cols): the AllReduce gates on
            # a quarter of the stage-in, and the rest streams during it.
            for k in range(kt):
                nc.sync.dma_start(
                    xstage[k][:, 0:xsh_cols], xT[k * 128:(k + 1) * 128, 0:xsh_cols]
                )
                nc.vector.tensor_reduce(
                    macc[:, k:k + 1], xstage[k][:, 0:xsh_cols],
                    axis=mybir.AxisListType.X,
                    op=mybir.AluOpType.max, apply_absolute_value=True,
                )
            for k in range(kt):
                ws = stage.tile([128, wsh_cols], F32, tag="ws")
                nc.sync.dma_start(ws[:], wsh[k * 128:(k + 1) * 128, :])
                nc.vector.tensor_reduce(
                    macc[:, kt + k:kt + k + 1], ws[:], axis=mybir.AxisListType.X,
                    op=mybir.AluOpType.max, apply_absolute_value=True,
                )
            gm2 = small.tile([128, 2], F32, tag="gm2")
            nc.vector.tensor_reduce(
                gm2[:, 0:1], macc[:, 0:kt], axis=mybir.AxisListType.X,
                op=mybir.AluOpType.max,
            )
            nc.vector.tensor_reduce(
                gm2[:, 1:2], macc[:, kt:2 * kt], axis=mybir.AxisListType.X,
                op=mybir.AluOpType.max,
            )
            # partition-max via DRAM round-trip transposing gather (the custom
            # GPSIMD partition ops fail codegen in this walrus build)
            nc.sync.dma_start(scr[:, :], gm2[:])
            g3 = small.tile([1, 2, 128], F32, tag="g3")
            nc.sync.dma_start(g3[:], bass.AP(scr, 0, [[0, 1], [1, 2], [2, 128]]))
            lmax = small.tile([1, 2], F32, tag="lmax")
            nc.vector.tensor_reduce(
                lmax[:], g3[:], axis=mybir.AxisListType.X, op=mybir.AluOpType.max
            )
            # AllReduce(max) of [max|x|, max|W|] across the 8 cores
            cc_in_dma = nc.sync.dma_start(cc_in[:, :], lmax[:])
            nc.gpsimd.collective_compute(
                "AllReduce", mybir.AluOpType.max, replica_groups=groups,
                ins=[cc_in[:, :]], outs=[cc_out[:, :]],
            )
            gmx = small.tile([128, 2], F32, tag="gmx")
            nc.sync.dma_start(gmx[:], cc_out[0:1, :].broadcast_to([128, 2]))
            # stage the non-shard x columns during the collective window; the
            # explicit dep keeps the (serialized) DMA engines clear until the
            # tiny AllReduce input is on its way
            for k in range(kt):
                d = nc.sync.dma_start(
                    xstage[k][:, xsh_cols:m_core],
                    xT[k * 128:(k + 1) * 128, xsh_cols:m_core],
                )
                add_dep_helper(d.ins, cc_in_dma.ins, sync=True,
                               reason="keep DMA engines clear pre-AllReduce")
            # scales: s = gmax/127 (jnp computes max/127; mult by 1/127 is
            # within 1 ulp), inv = 1/s, ss = sx*sw
            sxsw = small.tile([128, 2], F32, tag="sxsw")
            nc.vector.tensor_scalar_mul(sxsw[:], gmx[:], 1.0 / 127.0)
            inv = small.tile([128, 2], F32, tag="inv")
            nc.vector.reciprocal(inv[:], sxsw[:])
            ss = small.tile([128, 1], F32, tag="ss")
            nc.vector.tensor_tensor(
                ss[:], sxsw[:, 0:1], sxsw[:, 1:2], op=mybir.AluOpType.mult
            )
            bt = small.tile([128, n_it], F32, tag="bt")
            nc.sync.dma_start(bt[:], bias[:, :])

            # ---- phase 1: quantize x from the staged chunks (no DMA) ----
            xq = res.tile([128, kt * m_core], BF16, tag="xq")  # [p, k, tok]
            wq = res.tile([128, kt * i_core], BF16, tag="wq")  # [p, k, I]
            qchunks = [(0, 512)] + [(c, 512) for c in range(512, m_core, 512)]
            for c0, cw in qchunks:
                for k in range(kt):
                    t1 = stage.tile([128, 512], F32, tag="t1", bufs=4)
                    nc.scalar.activation(
                        t1[:, :cw], xstage[k][:, c0:c0 + cw],
                        mybir.ActivationFunctionType.Copy,
                        bias=MAGIC, scale=inv[:, 0:1],
                    )
                    nc.vector.tensor_scalar(
                        xq[:, k * m_core + c0:k * m_core + c0 + cw],
                        t1[:, :cw], MAGIC, None, op0=mybir.AluOpType.subtract,
                    )

            # ---- phase 2: stream+quantize W by I-quads; matmul + gelu ----
            # Two passes over the I-tiles: pass A covers token group 0 only
            # (gated by just 1/4 of the x-quantize), pass B the remaining
            # groups once the quantizer has finished. W streams during pass A.
            IQ = 512  # W I-columns per streamed chunk = 4 output I-tiles

            def mm_evac(i, tgs):
                ps = [
                    pp.tile([128, 512], F32, tag="ps", name=f"ps_{i}_{tg}")
                    for tg in tgs
                ]
                for k in range(kt):
                    lhsT = wq[:, k * i_core + i * 128:k * i_core + (i + 1) * 128]
                    for j, tg in enumerate(tgs):
                        rhs = xq[:, k * m_core + tg * 512:
                                 k * m_core + (tg + 1) * 512]
                        nc.tensor.matmul(
                            ps[j][:], lhsT, rhs,
                            start=(k == 0), stop=(k == kt - 1),
                        )
                for j, tg in enumerate(tgs):
                    ot = evac.tile([128, 512], F32, tag="ot")
                    nc.scalar.activation(
                        ot[:], ps[j][:],
                        mybir.ActivationFunctionType.Gelu,
                        bias=bt[:, i:i + 1], scale=ss[:, 0:1],
                    )
                    nc.sync.dma_start(
                        outT[i * 128:(i + 1) * 128, tg * 512:(tg + 1) * 512],
                        ot[:],
                    )

            for q0 in range(0, i_core, IQ):
                qw = min(IQ, i_core - q0)
                for k in range(kt):
                    wf = stage.tile([128, IQ], F32, tag="wf", bufs=8)
                    d = nc.sync.dma_start(
                        wf[:, :qw], wT[k * 128:(k + 1) * 128, q0:q0 + qw]
                    )
                    add_dep_helper(d.ins, cc_in_dma.ins, sync=True,
                                   reason="keep DMA engines clear pre-AllReduce")
                    t2 = stage.tile([128, IQ], F32, tag="t2")
                    nc.vector.tensor_scalar(
                        t2[:, :qw], wf[:, :qw], inv[:, 1:2], MAGIC,
                        op0=mybir.AluOpType.mult, op1=mybir.AluOpType.add,
                    )
                    nc.vector.tensor_scalar(
                        wq[:, k * i_core + q0:k * i_core + q0 + qw],
                        t2[:, :qw], MAGIC, None, op0=mybir.AluOpType.subtract,
                    )
                for i in range(q0 // 128, (q0 + qw) // 128):
                    mm_evac(i, [0])
            for i in range(n_it):
                mm_evac(i, list(range(1, n_tg)))
    _split_sync_waits(nc)
    return nc


_CACHE: dict = {}


def _get_nc():
    if "nc" not in _CACHE:
        _CACHE["nc"] = build()
    return _CACHE["nc"]


def shard_inputs(x, W, b):
    """Host-side sharding: pure layout (transpose/slice/replicate), no math."""
    x2 = np.ascontiguousarray(x.reshape(M, H).T)  # [H, M]
    in_maps = []
    for c in range(N_CORES):
        ti, ii = c // II, c % II
        mq, ih = M // TI, I // II
        q = x2[:, ti * mq:(ti + 1) * mq]
        sh = mq // II
        perm = np.r_[ii * sh:(ii + 1) * sh, 0:ii * sh, (ii + 1) * sh:mq]
        xT = np.ascontiguousarray(q[:, perm])
        wT = np.ascontiguousarray(W[ii * ih:(ii + 1) * ih, :].T)
        # distinct 1/8 shard of W for the global max reduction (x maxes come
        # from the staged full quarter on-device, duplicated across core pairs)
        wss = np.ascontiguousarray(wT[:, ti * (ih // TI):(ti + 1) * (ih // TI)])
        bia = np.ascontiguousarray(
            b[ii * ih:(ii + 1) * ih].reshape(ih // 128, 128).T
        )
        in_maps.append({"xT": xT, "wT": wT, "wsh": wss, "bias": bia})
    return in_maps


def unshard_output(results):
    """Assemble per-core transposed blocks into the full [B, S, I] output."""
    outT = np.empty((I, M), np.float32)
    for c in range(N_CORES):
        ti, ii = c // II, c % II
        mq, ih = M // TI, I // II
        sh = mq // II
        perm = np.r_[ii * sh:(ii + 1) * sh, 0:ii * sh, (ii + 1) * sh:mq]
        inv_perm = np.argsort(perm)
        outT[ii * ih:(ii + 1) * ih, ti * mq:(ti + 1) * mq] = \
            results[c]["outT"][:, inv_perm]
    return np.ascontiguousarray(outT.T).reshape(B, S, I)


def kernel(x, W, b):
    nc = _get_nc()
    in_maps = shard_inputs(
        np.asarray(x, np.float32), np.asarray(W, np.float32), np.asarray(b, np.float32)
    )
    res = bass_utils.run_bass_kernel_spmd(nc, in_maps, core_ids=list(range(N_CORES)))
    return unshard_output(res.results)


# revision 26
# speedup vs baseline: 1.2132x; 1.0419x over previous
"""Trainium2 Bass kernel for nn_BertIntermediate (QuantizeLinear + exact GELU).

Reference computation (see harness reference):
    xq = fake_quant(x)   # symmetric per-tensor int8 fake quant, scale = max|x|/127
    Wq = fake_quant(W)
    h  = xq @ Wq.T + b
    out = h * 0.5 * (1 + erf(h/sqrt(2)))

Key numerical insight: q = round(v/scale) is an integer in [-127, 127], exactly
representable in bf16 (8-bit mantissa holds integers up to 256). Products are
<= 127*128 and k-sums over H=1024 stay below 2^24, so a bf16 matmul with fp32
PSUM accumulation reproduces the fp32 reference EXACTLY (up to rounding-tie
flips worth ~1e-3 absolute). The scales factor out of the GEMM:
    h = (sx*sw) * (qx @ qW.T) + b
and fold into the scalar-engine activation (gelu(scale*psum + bias)).

Sharding (8 cores): 2D grid, 4-way over tokens x 2-way over intermediate dim.
Per core: x^T quarter [1024, 2048] replicated x2, W^T half [1024, 2048]
replicated x4, output block written transposed [2048 I, 2048 tok].
This costs ~34 MB DMA per core vs ~53 MB for the pure Megatron column split,
balancing DMA (~100us) against PE (~110us) at the roofline ridge.

The global quantization scales need max|x|, max|W| over the FULL tensors: each
core reduces a distinct 1/8 shard (passed as extra inputs), then a tiny
AllReduce(max) of 2 floats across the 8 cores combines them on-device.

Rounding: round-half-to-even via the fp32 magic-number trick
    rne(v) = (v + 1.5*2^23) - 1.5*2^23        (exact for |v| <= 2^22)
which matches jnp.round's banker's rounding.
"""

import numpy as np

import concourse.bass as bass
import concourse.bass_isa as bass_isa
import concourse.mybir as mybir
from concourse import bass_utils
from concourse.tile import TileContext
from concourse.tile_rust import add_dep_helper

F32 = mybir.dt.float32
BF16 = mybir.dt.bfloat16
MAGIC = 12582912.0  # 1.5 * 2**23: fp32 add/sub rounds to nearest int (RNE)
N_CORES = 8
TI, II = 4, 2  # token-quarters x intermediate-halves

# Full problem dims
B, S, H, I = 16, 512, 1024, 4096
M = B * S  # 8192 tokens


def _split_sync_waits(nc, max_waits=1):
    """Walrus in this container rejects instructions carrying more than a
    couple of sync-wait commands ("Too many sync wait commands"). Hoist excess
    waits onto single-wait nops inserted just before the instruction on the
    same engine queue — sequencers process in order, so semantics are
    unchanged."""
    n = 0
    for fn in nc.m.functions:
        for blk in fn.blocks:
            new_insts = []
            for inst in blk.instructions:
                si = inst.sync_info
                waits = list(si.on_wait or []) if si is not None else []
                if len(waits) > max_waits:
                    keep = waits[-max_waits:]
                    for w in waits[:-max_waits]:
                        n += 1
                        nop = mybir.InstNoOp(
                            name=f"I-waitsplit-{n}",
                            ins=[],
                            outs=[],
                            engine=inst.engine,
                        )
                        nop.sync_info = mybir.SyncInfo(on_wait=[w], on_update=[])
                        new_insts.append(nop)
                    inst.sync_info = mybir.SyncInfo(
                        on_wait=keep, on_update=list(si.on_update or [])
                    )
                new_insts.append(inst)
            blk.instructions = new_insts


def build(h=H, m_core=M // TI, i_core=I // II, xsh_cols=None, wsh_cols=None):
    """Build the SPMD Bass program for one core's block.

    h:      contraction dim (multiple of 128)
    m_core: tokens per core (multiple of 512)
    i_core: intermediate outputs per core (multiple of 128)
    """
    if xsh_cols is None:
        xsh_cols = m_core // II
    if wsh_cols is None:
        wsh_cols = i_core // TI
    kt = h // 128          # contraction tiles
    n_it = i_core // 128   # output I-tiles (PSUM partition dim)
    n_tg = m_core // 512   # token groups (PSUM free dim)
    n_bt = (n_it + 15) // 16  # bias tile columns padded into [128, n_it]

    nc = bass.Bass(num_devices=N_CORES)
    xT = nc.dram_tensor("xT", [h, m_core], F32, kind="ExternalInput")
    wT = nc.dram_tensor("wT", [h, i_core], F32, kind="ExternalInput")
    bias = nc.dram_tensor("bias", [128, n_it], F32, kind="ExternalInput")
    outT = nc.dram_tensor("outT", [i_core, m_core], F32, kind="ExternalOutput")
    cc_in = nc.dram_tensor("cc_in", [1, 256], F32, kind="Internal")
    cc_out = nc.dram_tensor("cc_out", [1, 256], F32, kind="Internal", addr_space="Shared")
    scr = nc.dram_tensor("scr", [128, 2], F32, kind="Internal")

    groups = [list(range(N_CORES))]

    with TileContext(nc) as tc:
        with (
            tc.tile_pool(name="res", bufs=1) as res,
            tc.tile_pool(name="stage", bufs=3) as stage,
            tc.tile_pool(name="small", bufs=1) as small,
            tc.tile_pool(name="psum", bufs=8, space="PSUM") as pp,
            tc.tile_pool(name="evac", bufs=4) as evac,
        ):
            # ---- phase 0: stage x fully; local maxes; AllReduce; scales ----
            # x chunks stage in SBUF during the reduction/collective so the
            # quantize pass after the scales arrive does no DMA. Core pairs
            # reduce the same token quarter (duplicated coverage is fine for a
            # max); W gets a distinct per-core 1/8 shard input instead since
            # its half streams only later during the matmul phase.
            xstage_all = res.tile([128, kt * m_core], F32, tag="xstage")
            xstage = [
                xstage_all[:, k * m_core:(k + 1) * m_core] for k in range(kt)
            ]
            macc = small.tile([128, 2 * kt], F32, tag="macc")
            # The host permutes each core's token columns so its distinct 1/8
            # max-shard sits in columns [0, xsh_cols): the AllReduce gates on
            # a quarter of the stage-in, and the rest streams during it.
            ng = kt  # staging DMA granularity: per k-tile for tight pipelining
            kg = kt // ng
            for g in range(ng):
                dst = xstage_all.rearrange(
                    "p (k c) -> p k c", k=kt
                )[:, g * kg:(g + 1) * kg, 0:xsh_cols]
                srcap = bass.AP(
                    xT, g * kg * 128 * m_core,
                    [[m_core, 128], [128 * m_core, kg], [1, xsh_cols]],
                )
                nc.sync.dma_start(dst, srcap)
                nc.vector.tensor_reduce(
                    macc[:, g * kg:(g + 1) * kg], dst,
                    axis=mybir.AxisListType.X,
                    op=mybir.AluOpType.max, apply_absolute_value=True,
                )
            # W shard = I-quad 0 of the (host-permuted) W half; its prefetch
            # doubles as the max-reduce input and is quantized after the AR
            wf0 = [
                stage.tile([128, 512], F32, tag="wf0", bufs=kt, name=f"wf0_{k}")
                for k in range(kt)
            ]
            for k in range(kt):
                nc.sync.dma_start(wf0[k][:], wT[k * 128:(k + 1) * 128, 0:512])
                nc.vector.tensor_reduce(
                    macc[:, kt + k:kt + k + 1], wf0[k][:],
                    axis=mybir.AxisListType.X,
                    op=mybir.AluOpType.max, apply_absolute_value=True,
                )
            gm2 = small.tile([128, 2], F32, tag="gm2")
            nc.vector.tensor_reduce(
                gm2[:, 0:1], macc[:, 0:kt], axis=mybir.AxisListType.X,
                op=mybir.AluOpType.max,
            )
            nc.vector.tensor_reduce(
                gm2[:, 1:2], macc[:, kt:2 * kt], axis=mybir.AxisListType.X,
                op=mybir.AluOpType.max,
            )
            # partition-max via DRAM round trip (the custom GPSIMD partition
            # ops fail codegen in this walrus build): gm2 -> scr, then a
            # DRAM->DRAM transposing gather lays the 256 values out c-major in
            # cc_in; the AllReduce maxes all 256 lanes and the cross-partition
            # reduction happens after broadcast, off the critical path.
            nc.sync.dma_start(scr[:, :], gm2[:])
            with nc.allow_non_contiguous_dma(reason="256-element gather"):
                cc_in_dma = nc.sync.dma_start(
                    bass.AP(cc_in, 0, [[256, 1], [128, 2], [1, 128]]),
                    bass.AP(scr, 0, [[0, 1], [1, 2], [2, 128]]),
                )
            nc.gpsimd.collective_compute(
                "AllReduce", mybir.AluOpType.max, replica_groups=groups,
                ins=[cc_in[:, :]], outs=[cc_out[:, :]],
            )
            g4 = small.tile([128, 256], F32, tag="g4")
            nc.sync.dma_start(g4[:], cc_out[0:1, :].broadcast_to([128, 256]))
            gmx = small.tile([128, 2], F32, tag="gmx")
            nc.vector.tensor_reduce(
                gmx[:], g4[:, :].rearrange("p (a b) -> p a b", a=2),
                axis=mybir.AxisListType.X, op=mybir.AluOpType.max,
            )
            # stage the non-shard x columns during the collective window; the
            # explicit dep keeps the (serialized) DMA engines clear until the
            # tiny AllReduce input is on its way
            rw = m_core - xsh_cols
            for g in range(ng):
                dst = xstage_all.rearrange(
                    "p (k c) -> p k c", k=kt
                )[:, g * kg:(g + 1) * kg, xsh_cols:m_core]
                srcap = bass.AP(
                    xT, g * kg * 128 * m_core + xsh_cols,
                    [[m_core, 128], [128 * m_core, kg], [1, rw]],
                )
                d = nc.sync.dma_start(dst, srcap)
                add_dep_helper(d.ins, cc_in_dma.ins, sync=True,
                               reason="keep DMA engines clear pre-AllReduce")
            # scales: s = gmax/127 (jnp computes max/127; mult by 1/127 is
            # within 1 ulp), inv = 1/s, ss = sx*sw
            sxsw = small.tile([128, 2], F32, tag="sxsw")
            nc.vector.tensor_scalar_mul(sxsw[:], gmx[:], 1.0 / 127.0)
            inv = small.tile([128, 2], F32, tag="inv")
            nc.vector.reciprocal(inv[:], sxsw[:])
            ss = small.tile([128, 1], F32, tag="ss")
            nc.vector.tensor_tensor(
                ss[:], sxsw[:, 0:1], sxsw[:, 1:2], op=mybir.AluOpType.mult
            )
            bt = small.tile([128, n_it], F32, tag="bt")
            nc.sync.dma_start(bt[:], bias[:, :])

            # ---- phase 1: quantize x from the staged chunks (no DMA) ----
            xq = res.tile([128, kt * m_core], BF16, tag="xq")  # [p, k, tok]
            wq = res.tile([128, kt * i_core], BF16, tag="wq")  # [p, k, I]
            qchunks = [(c, 512) for c in range(0, m_core, 512)]
            for c0, cw in qchunks:
                for k in range(kt):
                    t1 = stage.tile([128, 512], F32, tag="t1", bufs=4)
                    nc.scalar.activation(
                        t1[:, :cw], xstage[k][:, c0:c0 + cw],
                        mybir.ActivationFunctionType.Copy,
                        bias=MAGIC, scale=inv[:, 0:1],
                    )
                    nc.vector.tensor_scalar(
                        xq[:, k * m_core + c0:k * m_core + c0 + cw],
                        t1[:, :cw], MAGIC, None, op0=mybir.AluOpType.subtract,
                    )

            # ---- phase 2: stream+quantize W by I-quads; matmul + gelu ----
            # Two passes over the I-tiles: pass A covers token group 0 only
            # (gated by just 1/4 of the x-quantize), pass B the remaining
            # groups once the quantizer has finished. W streams during pass A.
            IQ = 512  # W I-columns per streamed chunk = 4 output I-tiles

            def mm_evac(i, tgs):
                ps = [
                    pp.tile([128, 512], F32, tag="ps", name=f"ps_{i}_{tg}")
                    for tg in tgs
                ]
                for k in range(kt):
                    lhsT = wq[:, k * i_core + i * 128:k * i_core + (i + 1) * 128]
                    for j, tg in enumerate(tgs):
                        rhs = xq[:, k * m_core + tg * 512:
                                 k * m_core + (tg + 1) * 512]
                        nc.tensor.matmul(
                            ps[j][:], lhsT, rhs,
                            start=(k == 0), stop=(k == kt - 1),
                        )
                for j, tg in enumerate(tgs):
                    ot = evac.tile([128, 512], F32, tag="ot")
                    nc.scalar.activation(
                        ot[:], ps[j][:],
                        mybir.ActivationFunctionType.Gelu,
                        bias=bt[:, i:i + 1], scale=ss[:, 0:1],
                    )
                    nc.sync.dma_start(
                        outT[i * 128:(i + 1) * 128, tg * 512:(tg + 1) * 512],
                        ot[:],
                    )

            for q0 in range(0, i_core, IQ):
                qw = min(IQ, i_core - q0)
                for k in range(kt):
                    if q0 == 0:
                        wf = wf0[k]
                    else:
                        wf = stage.tile([128, IQ], F32, tag="wf", bufs=8)
                        d = nc.sync.dma_start(
                            wf[:, :qw], wT[k * 128:(k + 1) * 128, q0:q0 + qw]
                        )
                        add_dep_helper(
                            d.ins, cc_in_dma.ins, sync=True,
                            reason="keep DMA engines clear pre-AllReduce")
                    t2 = stage.tile([128, IQ], F32, tag="t2")
                    nc.vector.tensor_scalar(
                        t2[:, :qw], wf[:, :qw], inv[:, 1:2], MAGIC,
                        op0=mybir.AluOpType.mult, op1=mybir.AluOpType.add,
                    )
                    nc.vector.tensor_scalar(
                        wq[:, k * i_core + q0:k * i_core + q0 + qw],
                        t2[:, :qw], MAGIC, None, op0=mybir.AluOpType.subtract,
                    )
                for i in range(q0 // 128, (q0 + qw) // 128):
                    mm_evac(i, [0, 1])
            for tg in range(2, n_tg):
                for i in range(n_it):
                    mm_evac(i, [tg])
    _split_sync_waits(nc)
    return nc


_CACHE: dict = {}


def _get_nc():
    if "nc" not in _CACHE:
        _CACHE["nc"] = build()
    return _CACHE["nc"]


def shard_inputs(x, W, b):
    """Host-side sharding: pure layout (transpose/slice/replicate), no math."""
    x2 = np.ascontiguousarray(x.reshape(M, H).T)  # [H, M]
    in_maps = []
    for c in range(N_CORES):
        ti, ii = c // II, c % II
        mq, ih = M // TI, I // II
        q = x2[:, ti * mq:(ti + 1) * mq]
        sh = mq // II
        perm = np.r_[ii * sh:(ii + 1) * sh, 0:ii * sh, (ii + 1) * sh:mq]
        xT = np.ascontiguousarray(q[:, perm])
        # permute W columns so this core's distinct 1/8 max-shard (an I-quad)
        # is block 0: its prefetch doubles as the shard max input
        wsd = ih // TI
        wperm = np.r_[ti * wsd:(ti + 1) * wsd, 0:ti * wsd, (ti + 1) * wsd:ih]
        wT = np.ascontiguousarray(W[ii * ih:(ii + 1) * ih, :].T[:, wperm])
        bia = np.ascontiguousarray(
            b[ii * ih:(ii + 1) * ih][wperm].reshape(ih // 128, 128).T
        )
        in_maps.append({"xT": xT, "wT": wT, "bias": bia})
    return in_maps


def unshard_output(results):
    """Assemble per-core transposed blocks into the full [B, S, I] output."""
    outT = np.empty((I, M), np.float32)
    for c in range(N_CORES):
        ti, ii = c // II, c % II
        mq, ih = M // TI, I // II
        sh = mq // II
        perm = np.r_[ii * sh:(ii + 1) * sh, 0:ii * sh, (ii + 1) * sh:mq]
        wsd = ih // TI
        wperm = np.r_[ti * wsd:(ti + 1) * wsd, 0:ti * wsd, (ti + 1) * wsd:ih]
        outT[ii * ih:(ii + 1) * ih, ti * mq:(ti + 1) * mq] = \
            results[c]["outT"][np.argsort(wperm)][:, np.argsort(perm)]
    return np.ascontiguousarray(outT.T).reshape(B, S, I)


def kernel(x, W, b):
    nc = _get_nc()
    in_maps = shard_inputs(
        np.asarray(x, np.float32), np.asarray(W, np.float32), np.asarray(b, np.float32)
    )
    res = bass_utils.run_bass_kernel_spmd(nc, in_maps, core_ids=list(range(N_CORES)))
    return unshard_output(res.results)


# revision 42
# speedup vs baseline: 1.3217x; 1.0894x over previous
"""Trainium2 Bass kernel for nn_BertIntermediate (QuantizeLinear + exact GELU).

Reference computation (see harness reference):
    xq = fake_quant(x)   # symmetric per-tensor int8 fake quant, scale = max|x|/127
    Wq = fake_quant(W)
    h  = xq @ Wq.T + b
    out = h * 0.5 * (1 + erf(h/sqrt(2)))

Key numerical insight: q = round(v/scale) is an integer in [-127, 127], exactly
representable in bf16 (8-bit mantissa holds integers up to 256). Products are
<= 127*128 and k-sums over H=1024 stay below 2^24, so a bf16 matmul with fp32
PSUM accumulation reproduces the fp32 reference EXACTLY (up to rounding-tie
flips worth ~1e-3 absolute). The scales factor out of the GEMM:
    h = (sx*sw) * (qx @ qW.T) + b
and fold into the scalar-engine activation (gelu(scale*psum + bias)).

Sharding (8 cores): 2D grid, 4-way over tokens x 2-way over intermediate dim.
Per core: x^T quarter [1024, 2048] replicated x2, W^T half [1024, 2048]
replicated x4, output block written transposed [2048 I, 2048 tok].
This costs ~34 MB DMA per core vs ~53 MB for the pure Megatron column split,
balancing DMA (~100us) against PE (~110us) at the roofline ridge.

The global quantization scales need max|x|, max|W| over the FULL tensors: each
core reduces a distinct 1/8 shard (the host permutes token/I columns so every
core's shard sits in block 0 of its inputs), a PE-transpose folds the
per-partition maxes, and a tiny padded AllGather + local max across the 8
cores yields the global scales on-device.

Rounding: round-half-to-even via the fp32 magic-number trick
    rne(v) = (v + 1.5*2^23) - 1.5*2^23        (exact for |v| <= 2^22)
which matches jnp.round's banker's rounding.
"""

import numpy as np

import concourse.bass as bass
import concourse.mybir as mybir
from concourse import bass_utils
from concourse.tile import TileContext
from concourse.tile_rust import add_dep_helper

F32 = mybir.dt.float32
BF16 = mybir.dt.bfloat16
MAGIC = 12582912.0  # 1.5 * 2**23: fp32 add/sub rounds to nearest int (RNE)
N_CORES = 8
TI, II = 4, 2  # token-quarters x intermediate-halves

# Full problem dims
B, S, H, I = 16, 512, 1024, 4096
M = B * S  # 8192 tokens


def _split_sync_waits(nc, max_waits=1):
    """Walrus in this container rejects instructions carrying more than a
    couple of sync-wait commands ("Too many sync wait commands"). Hoist excess
    waits onto single-wait nops inserted just before the instruction on the
    same engine queue — sequencers process in order, so semantics are
    unchanged."""
    n = 0
    for fn in nc.m.functions:
        for blk in fn.blocks:
            new_insts = []
            for inst in blk.instructions:
                si = inst.sync_info
                waits = list(si.on_wait or []) if si is not None else []
                if len(waits) > max_waits:
                    keep = waits[-max_waits:]
                    for w in waits[:-max_waits]:
                        n += 1
                        nop = mybir.InstNoOp(
                            name=f"I-waitsplit-{n}",
                            ins=[],
                            outs=[],
                            engine=inst.engine,
                        )
                        nop.sync_info = mybir.SyncInfo(on_wait=[w], on_update=[])
                        new_insts.append(nop)
                    inst.sync_info = mybir.SyncInfo(
                        on_wait=keep, on_update=list(si.on_update or [])
                    )
                new_insts.append(inst)
            blk.instructions = new_insts


def build(h=H, m_core=M // TI, i_core=I // II, xsh_cols=None, wsh_cols=None):
    """Build the SPMD Bass program for one core's block.

    h:      contraction dim (multiple of 128)
    m_core: tokens per core (multiple of 512)
    i_core: intermediate outputs per core (multiple of 128)
    """
    if xsh_cols is None:
        xsh_cols = m_core // II
    if wsh_cols is None:
        wsh_cols = i_core // TI
    kt = h // 128          # contraction tiles
    n_it = i_core // 128   # output I-tiles (PSUM partition dim)
    n_tg = m_core // 512   # token groups (PSUM free dim)

    nc = bass.Bass(num_devices=N_CORES)
    xT = nc.dram_tensor("xT", [h, m_core], F32, kind="ExternalInput")
    wT = nc.dram_tensor("wT", [h, i_core], F32, kind="ExternalInput")
    bias = nc.dram_tensor("bias", [128, n_it], F32, kind="ExternalInput")
    outT = nc.dram_tensor("outT", [i_core, m_core], F32, kind="ExternalOutput")
    CCW = 16  # payload padded to 64 B/rank; only the first 2 floats are used
    cc_in = nc.dram_tensor("cc_in", [1, CCW], F32, kind="Internal")
    cc_out = nc.dram_tensor("cc_out", [1, CCW * N_CORES], F32, kind="Internal",
                            addr_space="Shared")
    ident = nc.inline_tensor(np.eye(128, dtype=np.float32), name="ident128")

    groups = [list(range(N_CORES))]

    with TileContext(nc) as tc:
        with (
            tc.tile_pool(name="res", bufs=1) as res,
            tc.tile_pool(name="stage", bufs=3) as stage,
            tc.tile_pool(name="small", bufs=1) as small,
            tc.tile_pool(name="psum", bufs=8, space="PSUM") as pp,
            tc.tile_pool(name="evac", bufs=4) as evac,
        ):
            # ---- phase 0: stage x fully; local maxes; AllReduce; scales ----
            # x chunks stage in SBUF during the reduction/collective so the
            # quantize pass after the scales arrive does no DMA. Core pairs
            # reduce the same token quarter (duplicated coverage is fine for a
            # max); W gets a distinct per-core 1/8 shard input instead since
            # its half streams only later during the matmul phase.
            xstage_all = res.tile([128, kt * m_core], F32, tag="xstage")
            xstage = [
                xstage_all[:, k * m_core:(k + 1) * m_core] for k in range(kt)
            ]
            macc = small.tile([128, 2 * kt], F32, tag="macc")
            # The host permutes each core's token columns so its distinct 1/8
            # max-shard sits in columns [0, xsh_cols): the AllReduce gates on
            # a quarter of the stage-in, and the rest streams during it.
            ng = kt  # staging DMA granularity: per k-tile for tight pipelining
            kg = kt // ng
            for g in range(ng):
                dst = xstage_all.rearrange(
                    "p (k c) -> p k c", k=kt
                )[:, g * kg:(g + 1) * kg, 0:xsh_cols]
                srcap = bass.AP(
                    xT, g * kg * 128 * m_core,
                    [[m_core, 128], [128 * m_core, kg], [1, xsh_cols]],
                )
                nc.sync.dma_start(dst, srcap)
                nc.vector.tensor_reduce(
                    macc[:, g * kg:(g + 1) * kg], dst,
                    axis=mybir.AxisListType.X,
                    op=mybir.AluOpType.max, apply_absolute_value=True,
                )
            # W shard = I-quad 0 of the (host-permuted) W half; its prefetch
            # doubles as the max-reduce input and is quantized after the AR
            wf0 = [
                stage.tile([128, 512], F32, tag="wf0", bufs=kt, name=f"wf0_{k}")
                for k in range(kt)
            ]
            for k in range(kt):
                nc.sync.dma_start(wf0[k][:], wT[k * 128:(k + 1) * 128, 0:512])
                nc.vector.tensor_reduce(
                    macc[:, kt + k:kt + k + 1], wf0[k][:],
                    axis=mybir.AxisListType.X,
                    op=mybir.AluOpType.max, apply_absolute_value=True,
                )
            gm2 = small.tile([128, 2], F32, tag="gm2")
            nc.vector.tensor_reduce(
                gm2[:, 0:1], macc[:, 0:kt], axis=mybir.AxisListType.X,
                op=mybir.AluOpType.max,
            )
            nc.vector.tensor_reduce(
                gm2[:, 1:2], macc[:, kt:2 * kt], axis=mybir.AxisListType.X,
                op=mybir.AluOpType.max,
            )
            # partition-max via PE transpose (the custom GPSIMD partition ops
            # fail codegen in this walrus build): gm2^T to PSUM, reduce the
            # free dim, then ship the 2 scalars to the collective
            idt = small.tile([128, 128], F32, tag="idt")
            nc.sync.dma_start(idt[:], ident[:, :])
            gm2t = pp.tile([2, 128], F32, tag="ps", name="gm2t")
            nc.tensor.transpose(gm2t[:], gm2[:], idt[:])
            lmax = small.tile([2, 1], F32, tag="lmax")
            nc.vector.tensor_reduce(
                lmax[:], gm2t[:], axis=mybir.AxisListType.X,
                op=mybir.AluOpType.max,
            )
            cc_in_dma = nc.sync.dma_start(cc_in[0:1, 0:2], lmax[0:2, 0:1])
            czero = small.tile([1, CCW], F32, tag="czero", name="czero")
            nc.vector.memset(czero[:], 0.0)
            nc.sync.dma_start(cc_in[0:1, 2:CCW], czero[0:1, 2:CCW])
            # AllGather + local max: the cost model (and HW) charges an
            # AllReduce ~1.9x an AllGather at this size
            nc.gpsimd.collective_compute(
                "AllGather", mybir.AluOpType.bypass, replica_groups=groups,
                ins=[cc_in[:, :]], outs=[cc_out[:, :]],
            )
            g5 = small.tile([128, 2 * N_CORES], F32, tag="g5")
            g5v = g5[:, :].rearrange("p (a b) -> p a b", a=2)
            for c in range(2):
                gmx_dma = nc.sync.dma_start(
                    g5[:, c * N_CORES:(c + 1) * N_CORES],
                    bass.AP(cc_out, c, [[0, 128], [CCW, N_CORES]]),
                )
            gmx = small.tile([128, 2], F32, tag="gmx")
            nc.vector.tensor_reduce(
                gmx[:], g5v, axis=mybir.AxisListType.X, op=mybir.AluOpType.max
            )
            # stage the non-shard x columns during the collective window; the
            # explicit dep keeps the (serialized) DMA engines clear until the
            # tiny AllReduce input is on its way
            rw = m_core - xsh_cols
            for g in range(ng):
                dst = xstage_all.rearrange(
                    "p (k c) -> p k c", k=kt
                )[:, g * kg:(g + 1) * kg, xsh_cols:m_core]
                srcap = bass.AP(
                    xT, g * kg * 128 * m_core + xsh_cols,
                    [[m_core, 128], [128 * m_core, kg], [1, rw]],
                )
                d = nc.sync.dma_start(dst, srcap)
                gate0 = cc_in_dma if g < ng - 1 else gmx_dma
                add_dep_helper(gate0 and d.ins, gate0.ins, sync=True,
                               reason="keep DMA engines clear pre-AllReduce")
            # scales: s = gmax/127 (jnp computes max/127; mult by 1/127 is
            # within 1 ulp), inv = 1/s, ss = sx*sw
            sxsw = small.tile([128, 2], F32, tag="sxsw")
            nc.vector.tensor_scalar_mul(sxsw[:], gmx[:], 1.0 / 127.0)
            inv = small.tile([128, 2], F32, tag="inv")
            nc.vector.reciprocal(inv[:], sxsw[:])
            ss = small.tile([128, 1], F32, tag="ss")
            nc.vector.tensor_tensor(
                ss[:], sxsw[:, 0:1], sxsw[:, 1:2], op=mybir.AluOpType.mult
            )
            bt = small.tile([128, n_it], F32, tag="bt")
            nc.sync.dma_start(bt[:], bias[:, :])

            # ---- phase 1: quantize x from the staged chunks (no DMA) ----
            xq = res.tile([128, kt * m_core], BF16, tag="xq")  # [p, k, tok]
            wq = res.tile([128, kt * i_core], BF16, tag="wq")  # [p, k, I]
            qchunks = [(c, 512) for c in range(0, m_core, 512)]
            for c0, cw in qchunks:
                for k in range(kt):
                    t1 = stage.tile([128, 512], F32, tag="t1", bufs=6)
                    nc.scalar.activation(
                        t1[:, :cw], xstage[k][:, c0:c0 + cw],
                        mybir.ActivationFunctionType.Copy,
                        bias=MAGIC, scale=inv[:, 0:1],
                    )
                    nc.vector.tensor_scalar(
                        xq[:, k * m_core + c0:k * m_core + c0 + cw],
                        t1[:, :cw], MAGIC, None, op0=mybir.AluOpType.subtract,
                    )

            # ---- phase 2: stream+quantize W by I-quads; matmul + gelu ----
            # Two passes over the I-tiles: pass A covers token group 0 only
            # (gated by just 1/4 of the x-quantize), pass B the remaining
            # groups once the quantizer has finished. W streams during pass A.
            IQ = 512  # W I-columns per streamed chunk = 4 output I-tiles

            def mm_evac(i, tgs):
                ps = [
                    pp.tile([128, 512], F32, tag="ps", name=f"ps_{i}_{tg}")
                    for tg in tgs
                ]
                for k in range(kt):
                    lhsT = wq[:, k * i_core + i * 128:k * i_core + (i + 1) * 128]
                    for j, tg in enumerate(tgs):
                        rhs = xq[:, k * m_core + tg * 512:
                                 k * m_core + (tg + 1) * 512]
                        nc.tensor.matmul(
                            ps[j][:], lhsT, rhs,
                            start=(k == 0), stop=(k == kt - 1),
                        )
                for j, tg in enumerate(tgs):
                    ot = evac.tile([128, 512], F32, tag="ot")
                    nc.scalar.activation(
                        ot[:], ps[j][:],
                        mybir.ActivationFunctionType.Gelu,
                        bias=bt[:, i:i + 1], scale=ss[:, 0:1],
                    )
                    nc.sync.dma_start(
                        outT[i * 128:(i + 1) * 128, tg * 512:(tg + 1) * 512],
                        ot[:],
                    )

            for q0 in range(0, i_core, IQ):
                qw = min(IQ, i_core - q0)
                for k in range(kt):
                    if q0 == 0:
                        wf = wf0[k]
                    else:
                        wf = stage.tile([128, IQ], F32, tag="wf", bufs=8)
                        d = nc.sync.dma_start(
                            wf[:, :qw], wT[k * 128:(k + 1) * 128, q0:q0 + qw]
                        )
                        gate = gmx_dma
                        add_dep_helper(
                            d.ins, gate.ins, sync=True,
                            reason="keep DMA engines clear pre-AllReduce")
                    t2 = stage.tile([128, IQ], F32, tag="t2")
                    nc.vector.tensor_scalar(
                        t2[:, :qw], wf[:, :qw], inv[:, 1:2], MAGIC,
                        op0=mybir.AluOpType.mult, op1=mybir.AluOpType.add,
                    )
                    nc.vector.tensor_scalar(
                        wq[:, k * i_core + q0:k * i_core + q0 + qw],
                        t2[:, :qw], MAGIC, None, op0=mybir.AluOpType.subtract,
                    )
                for i in range(q0 // 128, (q0 + qw) // 128):
                    mm_evac(i, [0, 1])
            for tg in range(2, n_tg):
                for i in range(n_it):
                    mm_evac(i, [tg])
    _split_sync_waits(nc)
    return nc


_CACHE: dict = {}


def _get_nc():
    if "nc" not in _CACHE:
        _CACHE["nc"] = build()
    return _CACHE["nc"]


def shard_inputs(x, W, b):
    """Host-side sharding: pure layout (transpose/slice/replicate), no math."""
    x2 = np.ascontiguousarray(x.reshape(M, H).T)  # [H, M]
    in_maps = []
    for c in range(N_CORES):
        ti, ii = c // II, c % II
        mq, ih = M // TI, I // II
        q = x2[:, ti * mq:(ti + 1) * mq]
        sh = mq // II
        perm = np.r_[ii * sh:(ii + 1) * sh, 0:ii * sh, (ii + 1) * sh:mq]
        xT = np.ascontiguousarray(q[:, perm])
        # permute W columns so this core's distinct 1/8 max-shard (an I-quad)
        # is block 0: its prefetch doubles as the shard max input
        wsd = ih // TI
        wperm = np.r_[ti * wsd:(ti + 1) * wsd, 0:ti * wsd, (ti + 1) * wsd:ih]
        wT = np.ascontiguousarray(W[ii * ih:(ii + 1) * ih, :].T[:, wperm])
        bia = np.ascontiguousarray(
            b[ii * ih:(ii + 1) * ih][wperm].reshape(ih // 128, 128).T
        )
        in_maps.append({"xT": xT, "wT": wT, "bias": bia})
    return in_maps


def unshard_output(results):
    """Assemble per-core transposed blocks into the full [B, S, I] output."""
    outT = np.empty((I, M), np.float32)
    for c in range(N_CORES):
        ti, ii = c // II, c % II
        mq, ih = M // TI, I // II
        sh = mq // II
        perm = np.r_[ii * sh:(ii + 1) * sh, 0:ii * sh, (ii + 1) * sh:mq]
        wsd = ih // TI
        wperm = np.r_[ti * wsd:(ti + 1) * wsd, 0:ti * wsd, (ti + 1) * wsd:ih]
        outT[ii * ih:(ii + 1) * ih, ti * mq:(ti + 1) * mq] = \
            results[c]["outT"][np.argsort(wperm)][:, np.argsort(perm)]
    return np.ascontiguousarray(outT.T).reshape(B, S, I)


def kernel(x, W, b):
    nc = _get_nc()
    in_maps = shard_inputs(
        np.asarray(x, np.float32), np.asarray(W, np.float32), np.asarray(b, np.float32)
    )
    res = bass_utils.run_bass_kernel_spmd(nc, in_maps, core_ids=list(range(N_CORES)))
    return unshard_output(res.results)


# revision 46
# speedup vs baseline: 1.3249x; 1.0024x over previous
"""Trainium2 Bass kernel for nn_BertIntermediate (QuantizeLinear + exact GELU).

Reference computation (see harness reference):
    xq = fake_quant(x)   # symmetric per-tensor int8 fake quant, scale = max|x|/127
    Wq = fake_quant(W)
    h  = xq @ Wq.T + b
    out = h * 0.5 * (1 + erf(h/sqrt(2)))

Key numerical insight: q = round(v/scale) is an integer in [-127, 127], exactly
representable in bf16 (8-bit mantissa holds integers up to 256). Products are
<= 127*128 and k-sums over H=1024 stay below 2^24, so a bf16 matmul with fp32
PSUM accumulation reproduces the fp32 reference EXACTLY (up to rounding-tie
flips worth ~1e-3 absolute). The scales factor out of the GEMM:
    h = (sx*sw) * (qx @ qW.T) + b
and fold into the scalar-engine activation (gelu(scale*psum + bias)).

Sharding (8 cores): 2D grid, 4-way over tokens x 2-way over intermediate dim.
Per core: x^T quarter [1024, 2048] replicated x2, W^T half [1024, 2048]
replicated x4, output block written transposed [2048 I, 2048 tok].
This costs ~34 MB DMA per core vs ~53 MB for the pure Megatron column split,
balancing DMA (~100us) against PE (~110us) at the roofline ridge.

The global quantization scales need max|x|, max|W| over the FULL tensors: each
core reduces a distinct 1/8 shard (the host permutes token/I columns so every
core's shard sits in block 0 of its inputs), a PE-transpose folds the
per-partition maxes, and a tiny padded AllGather + local max across the 8
cores yields the global scales on-device.

Rounding: round-half-to-even via the fp32 magic-number trick
    rne(v) = (v + 1.5*2^23) - 1.5*2^23        (exact for |v| <= 2^22)
which matches jnp.round's banker's rounding.
"""

import numpy as np

import concourse.bass as bass
import concourse.mybir as mybir
from concourse import bass_utils
from concourse.tile import TileContext
from concourse.tile_rust import add_dep_helper

F32 = mybir.dt.float32
BF16 = mybir.dt.bfloat16
MAGIC = 12582912.0  # 1.5 * 2**23: fp32 add/sub rounds to nearest int (RNE)
N_CORES = 8
TI, II = 4, 2  # token-quarters x intermediate-halves

# Full problem dims
B, S, H, I = 16, 512, 1024, 4096
M = B * S  # 8192 tokens


def _split_sync_waits(nc, max_waits=1):
    """Walrus in this container rejects instructions carrying more than a
    couple of sync-wait commands ("Too many sync wait commands"). Hoist excess
    waits onto single-wait nops inserted just before the instruction on the
    same engine queue — sequencers process in order, so semantics are
    unchanged."""
    n = 0
    for fn in nc.m.functions:
        for blk in fn.blocks:
            new_insts = []
            for inst in blk.instructions:
                si = inst.sync_info
                waits = list(si.on_wait or []) if si is not None else []
                if len(waits) > max_waits:
                    keep = waits[-max_waits:]
                    for w in waits[:-max_waits]:
                        n += 1
                        nop = mybir.InstNoOp(
                            name=f"I-waitsplit-{n}",
                            ins=[],
                            outs=[],
                            engine=inst.engine,
                        )
                        nop.sync_info = mybir.SyncInfo(on_wait=[w], on_update=[])
                        new_insts.append(nop)
                    inst.sync_info = mybir.SyncInfo(
                        on_wait=keep, on_update=list(si.on_update or [])
                    )
                new_insts.append(inst)
            blk.instructions = new_insts


def build(h=H, m_core=M // TI, i_core=I // II, xsh_cols=None, wsh_cols=None):
    """Build the SPMD Bass program for one core's block.

    h:      contraction dim (multiple of 128)
    m_core: tokens per core (multiple of 512)
    i_core: intermediate outputs per core (multiple of 128)
    """
    if xsh_cols is None:
        xsh_cols = m_core // II
    if wsh_cols is None:
        wsh_cols = i_core // TI
    kt = h // 128          # contraction tiles
    n_it = i_core // 128   # output I-tiles (PSUM partition dim)
    n_tg = m_core // 512   # token groups (PSUM free dim)

    nc = bass.Bass(num_devices=N_CORES)
    xT = nc.dram_tensor("xT", [h, m_core], F32, kind="ExternalInput")
    wT = nc.dram_tensor("wT", [h, i_core], F32, kind="ExternalInput")
    bias = nc.dram_tensor("bias", [128, n_it], F32, kind="ExternalInput")
    outT = nc.dram_tensor("outT", [i_core, m_core], F32, kind="ExternalOutput")
    CCW = 16  # payload padded to 64 B/rank; only the first 2 floats are used
    cc_in = nc.dram_tensor("cc_in", [1, CCW], F32, kind="Internal")
    cc_out = nc.dram_tensor("cc_out", [1, CCW * N_CORES], F32, kind="Internal",
                            addr_space="Shared")
    ident = nc.inline_tensor(np.eye(128, dtype=np.float32), name="ident128")

    groups = [list(range(N_CORES))]

    with TileContext(nc) as tc:
        with (
            tc.tile_pool(name="res", bufs=1) as res,
            tc.tile_pool(name="stage", bufs=3) as stage,
            tc.tile_pool(name="small", bufs=1) as small,
            tc.tile_pool(name="psum", bufs=8, space="PSUM") as pp,
            tc.tile_pool(name="evac", bufs=4) as evac,
        ):
            # ---- phase 0: stage x fully; local maxes; AllReduce; scales ----
            # x chunks stage in SBUF during the reduction/collective so the
            # quantize pass after the scales arrive does no DMA. Core pairs
            # reduce the same token quarter (duplicated coverage is fine for a
            # max); W gets a distinct per-core 1/8 shard input instead since
            # its half streams only later during the matmul phase.
            xstage_all = res.tile([128, kt * m_core], F32, tag="xstage")
            xstage = [
                xstage_all[:, k * m_core:(k + 1) * m_core] for k in range(kt)
            ]
            macc = small.tile([128, 2 * kt], F32, tag="macc")
            # The host permutes each core's token columns so its distinct 1/8
            # max-shard sits in columns [0, xsh_cols): the AllReduce gates on
            # a quarter of the stage-in, and the rest streams during it.
            ng = kt  # staging DMA granularity: per k-tile for tight pipelining
            kg = kt // ng
            for g in range(ng):
                dst = xstage_all.rearrange(
                    "p (k c) -> p k c", k=kt
                )[:, g * kg:(g + 1) * kg, 0:xsh_cols]
                srcap = bass.AP(
                    xT, g * kg * 128 * m_core,
                    [[m_core, 128], [128 * m_core, kg], [1, xsh_cols]],
                )
                nc.sync.dma_start(dst, srcap)
                nc.vector.tensor_reduce(
                    macc[:, g * kg:(g + 1) * kg], dst,
                    axis=mybir.AxisListType.X,
                    op=mybir.AluOpType.max, apply_absolute_value=True,
                )
            # W shard = I-quad 0 of the (host-permuted) W half; its prefetch
            # doubles as the max-reduce input and is quantized after the AR
            wf0 = [
                stage.tile([128, 512], F32, tag="wf0", bufs=kt, name=f"wf0_{k}")
                for k in range(kt)
            ]
            for k in range(kt):
                nc.sync.dma_start(wf0[k][:], wT[k * 128:(k + 1) * 128, 0:512])
                nc.vector.tensor_reduce(
                    macc[:, kt + k:kt + k + 1], wf0[k][:],
                    axis=mybir.AxisListType.X,
                    op=mybir.AluOpType.max, apply_absolute_value=True,
                )
            gm2 = small.tile([128, 2], F32, tag="gm2")
            nc.vector.tensor_reduce(
                gm2[:, 0:1], macc[:, 0:kt], axis=mybir.AxisListType.X,
                op=mybir.AluOpType.max,
            )
            nc.vector.tensor_reduce(
                gm2[:, 1:2], macc[:, kt:2 * kt], axis=mybir.AxisListType.X,
                op=mybir.AluOpType.max,
            )
            # partition-max via PE transpose (the custom GPSIMD partition ops
            # fail codegen in this walrus build): gm2^T to PSUM, reduce the
            # free dim, then ship the 2 scalars to the collective
            idt = small.tile([128, 128], F32, tag="idt")
            nc.sync.dma_start(idt[:], ident[:, :])
            gm2t = pp.tile([2, 128], F32, tag="ps", name="gm2t")
            nc.tensor.transpose(gm2t[:], gm2[:], idt[:])
            lmax = small.tile([2, 1], F32, tag="lmax")
            nc.vector.tensor_reduce(
                lmax[:], gm2t[:], axis=mybir.AxisListType.X,
                op=mybir.AluOpType.max,
            )
            cc_in_dma = nc.sync.dma_start(cc_in[0:1, 0:2], lmax[0:2, 0:1])
            czero = small.tile([1, CCW], F32, tag="czero", name="czero")
            nc.vector.memset(czero[:], 0.0)
            nc.sync.dma_start(cc_in[0:1, 2:CCW], czero[0:1, 2:CCW])
            # AllGather + local max: the cost model (and HW) charges an
            # AllReduce ~1.9x an AllGather at this size
            nc.gpsimd.collective_compute(
                "AllGather", mybir.AluOpType.bypass, replica_groups=groups,
                ins=[cc_in[:, :]], outs=[cc_out[:, :]],
            )
            g5 = small.tile([128, 2 * N_CORES], F32, tag="g5")
            g5v = g5[:, :].rearrange("p (a b) -> p a b", a=2)
            for c in range(2):
                gmx_dma = nc.sync.dma_start(
                    g5[:, c * N_CORES:(c + 1) * N_CORES],
                    bass.AP(cc_out, c, [[0, 128], [CCW, N_CORES]]),
                )
            gmx = small.tile([128, 2], F32, tag="gmx")
            nc.vector.tensor_reduce(
                gmx[:], g5v, axis=mybir.AxisListType.X, op=mybir.AluOpType.max
            )
            # stage the non-shard x columns during the collective window; the
            # explicit dep keeps the (serialized) DMA engines clear until the
            # tiny AllReduce input is on its way
            rw = m_core - xsh_cols
            for g in range(ng):
                dst = xstage_all.rearrange(
                    "p (k c) -> p k c", k=kt
                )[:, g * kg:(g + 1) * kg, xsh_cols:m_core]
                srcap = bass.AP(
                    xT, g * kg * 128 * m_core + xsh_cols,
                    [[m_core, 128], [128 * m_core, kg], [1, rw]],
                )
                d = nc.sync.dma_start(dst, srcap)
                gate0 = cc_in_dma if g < ng - 1 else gmx_dma
                add_dep_helper(gate0 and d.ins, gate0.ins, sync=True,
                               reason="keep DMA engines clear pre-AllReduce")
            # scales: s = gmax/127 (jnp computes max/127; mult by 1/127 is
            # within 1 ulp), inv = 1/s, ss = sx*sw
            sxsw = small.tile([128, 2], F32, tag="sxsw")
            nc.vector.tensor_scalar_mul(sxsw[:], gmx[:], 1.0 / 127.0)
            inv = small.tile([128, 2], F32, tag="inv")
            nc.vector.reciprocal(inv[:], sxsw[:])
            ss = small.tile([128, 1], F32, tag="ss")
            nc.vector.tensor_tensor(
                ss[:], sxsw[:, 0:1], sxsw[:, 1:2], op=mybir.AluOpType.mult
            )
            bt = small.tile([128, n_it], F32, tag="bt")
            nc.sync.dma_start(bt[:], bias[:, :])
            # ---- phase 1: quantize x from the staged chunks (no DMA) ----
            xq = res.tile([128, kt * m_core], BF16, tag="xq")  # [p, k, tok]
            wq = res.tile([128, kt * i_core], BF16, tag="wq")  # [p, k, I]
            qchunks = [(c, 512) for c in range(0, m_core, 512)]
            late_quant = []  # tg2+ chunks: defer behind W-quant (see below)
            for c0, cw in qchunks:
                for k in range(kt):
                    t1 = stage.tile([128, 512], F32, tag="t1", bufs=6)
                    a = nc.scalar.activation(
                        t1[:, :cw], xstage[k][:, c0:c0 + cw],
                        mybir.ActivationFunctionType.Copy,
                        bias=MAGIC, scale=inv[:, 0:1],
                    )
                    if c0 >= 1024:
                        late_quant.append(a)
                    nc.vector.tensor_scalar(
                        xq[:, k * m_core + c0:k * m_core + c0 + cw],
                        t1[:, :cw], MAGIC, None, op0=mybir.AluOpType.subtract,
                    )

            # ---- phase 2: stream+quantize W by I-quads; matmul + gelu ----
            # Two passes over the I-tiles: pass A covers token group 0 only
            # (gated by just 1/4 of the x-quantize), pass B the remaining
            # groups once the quantizer has finished. W streams during pass A.
            IQ = 512  # W I-columns per streamed chunk = 4 output I-tiles

            def mm_evac(i, tgs):
                ps = [
                    pp.tile([128, 512], F32, tag="ps", name=f"ps_{i}_{tg}")
                    for tg in tgs
                ]
                for k in range(kt):
                    lhsT = wq[:, k * i_core + i * 128:k * i_core + (i + 1) * 128]
                    for j, tg in enumerate(tgs):
                        rhs = xq[:, k * m_core + tg * 512:
                                 k * m_core + (tg + 1) * 512]
                        nc.tensor.matmul(
                            ps[j][:], lhsT, rhs,
                            start=(k == 0), stop=(k == kt - 1),
                        )
                for j, tg in enumerate(tgs):
                    ot = evac.tile([128, 512], F32, tag="ot")
                    nc.scalar.activation(
                        ot[:], ps[j][:],
                        mybir.ActivationFunctionType.Gelu,
                        bias=bt[:, i:i + 1], scale=ss[:, 0:1],
                    )
                    nc.sync.dma_start(
                        outT[i * 128:(i + 1) * 128, tg * 512:(tg + 1) * 512],
                        ot[:],
                    )

            for q0 in range(0, i_core, IQ):
                qw = min(IQ, i_core - q0)
                for k in range(kt):
                    if q0 == 0:
                        wf = wf0[k]
                    else:
                        wf = stage.tile([128, IQ], F32, tag="wf", bufs=8)
                        d = nc.sync.dma_start(
                            wf[:, :qw], wT[k * 128:(k + 1) * 128, q0:q0 + qw]
                        )
                        gate = gmx_dma
                        add_dep_helper(
                            d.ins, gate.ins, sync=True,
                            reason="keep DMA engines clear pre-AllReduce")
                    t2 = stage.tile([128, IQ], F32, tag="t2")
                    nc.vector.tensor_scalar(
                        t2[:, :qw], wf[:, :qw], inv[:, 1:2], MAGIC,
                        op0=mybir.AluOpType.mult, op1=mybir.AluOpType.add,
                    )
                    wlast = nc.vector.tensor_scalar(
                        wq[:, k * i_core + q0:k * i_core + q0 + qw],
                        t2[:, :qw], MAGIC, None, op0=mybir.AluOpType.subtract,
                    )
                if q0 == i_core - IQ:
                    # token groups 2-3 aren't consumed until pass B (~55us
                    # later): keep their quantize off the DVE/ACT while the
                    # W stream feeds the first matmul pass
                    for a in late_quant:
                        add_dep_helper(a.ins, wlast.ins, sync=True,
                                       reason="defer tg2+ x-quant behind W")
                for i in range(q0 // 128, (q0 + qw) // 128):
                    mm_evac(i, [0, 1])
            for tg in range(2, n_tg):
                for i in range(n_it):
                    mm_evac(i, [tg])
    _split_sync_waits(nc)
    return nc


_CACHE: dict = {}


def _get_nc():
    if "nc" not in _CACHE:
        _CACHE["nc"] = build()
    return _CACHE["nc"]


def shard_inputs(x, W, b):
    """Host-side sharding: pure layout (transpose/slice/replicate), no math."""
    x2 = np.ascontiguousarray(x.reshape(M, H).T)  # [H, M]
    in_maps = []
    for c in range(N_CORES):
        ti, ii = c // II, c % II
        mq, ih = M // TI, I // II
        q = x2[:, ti * mq:(ti + 1) * mq]
        sh = mq // II
        perm = np.r_[ii * sh:(ii + 1) * sh, 0:ii * sh, (ii + 1) * sh:mq]
        xT = np.ascontiguousarray(q[:, perm])
        # permute W columns so this core's distinct 1/8 max-shard (an I-quad)
        # is block 0: its prefetch doubles as the shard max input
        wsd = ih // TI
        wperm = np.r_[ti * wsd:(ti + 1) * wsd, 0:ti * wsd, (ti + 1) * wsd:ih]
        wT = np.ascontiguousarray(W[ii * ih:(ii + 1) * ih, :].T[:, wperm])
        bia = np.ascontiguousarray(
            b[ii * ih:(ii + 1) * ih][wperm].reshape(ih // 128, 128).T
        )
        in_maps.append({"xT": xT, "wT": wT, "bias": bia})
    return in_maps


def unshard_output(results):
    """Assemble per-core transposed blocks into the full [B, S, I] output."""
    outT = np.empty((I, M), np.float32)
    for c in range(N_CORES):
        ti, ii = c // II, c % II
        mq, ih = M // TI, I // II
        sh = mq // II
        perm = np.r_[ii * sh:(ii + 1) * sh, 0:ii * sh, (ii + 1) * sh:mq]
        wsd = ih // TI
        wperm = np.r_[ti * wsd:(ti + 1) * wsd, 0:ti * wsd, (ti + 1) * wsd:ih]
        outT[ii * ih:(ii + 1) * ih, ti * mq:(ti + 1) * mq] = \
            results[c]["outT"][np.argsort(wperm)][:, np.argsort(perm)]
    return np.ascontiguousarray(outT.T).reshape(B, S, I)


def kernel(x, W, b):
    nc = _get_nc()
    in_maps = shard_inputs(
        np.asarray(x, np.float32), np.asarray(W, np.float32), np.asarray(b, np.float32)
    )
    res = bass_utils.run_bass_kernel_spmd(nc, in_maps, core_ids=list(range(N_CORES)))
    return unshard_output(res.results)


# revision 47
# speedup vs baseline: 1.3302x; 1.0040x over previous
"""Trainium2 Bass kernel for nn_BertIntermediate (QuantizeLinear + exact GELU).

Reference computation (see harness reference):
    xq = fake_quant(x)   # symmetric per-tensor int8 fake quant, scale = max|x|/127
    Wq = fake_quant(W)
    h  = xq @ Wq.T + b
    out = h * 0.5 * (1 + erf(h/sqrt(2)))

Key numerical insight: q = round(v/scale) is an integer in [-127, 127], exactly
representable in bf16 (8-bit mantissa holds integers up to 256). Products are
<= 127*128 and k-sums over H=1024 stay below 2^24, so a bf16 matmul with fp32
PSUM accumulation reproduces the fp32 reference EXACTLY (up to rounding-tie
flips worth ~1e-3 absolute). The scales factor out of the GEMM:
    h = (sx*sw) * (qx @ qW.T) + b
and fold into the scalar-engine activation (gelu(scale*psum + bias)).

Sharding (8 cores): 2D grid, 4-way over tokens x 2-way over intermediate dim.
Per core: x^T quarter [1024, 2048] replicated x2, W^T half [1024, 2048]
replicated x4, output block written transposed [2048 I, 2048 tok].
This costs ~34 MB DMA per core vs ~53 MB for the pure Megatron column split,
balancing DMA (~100us) against PE (~110us) at the roofline ridge.

The global quantization scales need max|x|, max|W| over the FULL tensors: each
core reduces a distinct 1/8 shard (the host permutes token/I columns so every
core's shard sits in block 0 of its inputs), a PE-transpose folds the
per-partition maxes, and a tiny padded AllGather + local max across the 8
cores yields the global scales on-device.

Rounding: round-half-to-even via the fp32 magic-number trick
    rne(v) = (v + 1.5*2^23) - 1.5*2^23        (exact for |v| <= 2^22)
which matches jnp.round's banker's rounding.
"""

import numpy as np

import concourse.bass as bass
import concourse.mybir as mybir
from concourse import bass_utils
from concourse.tile import TileContext
from concourse.tile_rust import add_dep_helper

F32 = mybir.dt.float32
BF16 = mybir.dt.bfloat16
MAGIC = 12582912.0  # 1.5 * 2**23: fp32 add/sub rounds to nearest int (RNE)
N_CORES = 8
TI, II = 4, 2  # token-quarters x intermediate-halves

# Full problem dims
B, S, H, I = 16, 512, 1024, 4096
M = B * S  # 8192 tokens


def _split_sync_waits(nc, max_waits=1):
    """Walrus in this container rejects instructions carrying more than a
    couple of sync-wait commands ("Too many sync wait commands"). Hoist excess
    waits onto single-wait nops inserted just before the instruction on the
    same engine queue — sequencers process in order, so semantics are
    unchanged."""
    n = 0
    for fn in nc.m.functions:
        for blk in fn.blocks:
            new_insts = []
            for inst in blk.instructions:
                si = inst.sync_info
                waits = list(si.on_wait or []) if si is not None else []
                if len(waits) > max_waits:
                    keep = waits[-max_waits:]
                    for w in waits[:-max_waits]:
                        n += 1
                        nop = mybir.InstNoOp(
                            name=f"I-waitsplit-{n}",
                            ins=[],
                            outs=[],
                            engine=inst.engine,
                        )
                        nop.sync_info = mybir.SyncInfo(on_wait=[w], on_update=[])
                        new_insts.append(nop)
                    inst.sync_info = mybir.SyncInfo(
                        on_wait=keep, on_update=list(si.on_update or [])
                    )
                new_insts.append(inst)
            blk.instructions = new_insts


def build(h=H, m_core=M // TI, i_core=I // II, xsh_cols=None, wsh_cols=None):
    """Build the SPMD Bass program for one core's block.

    h:      contraction dim (multiple of 128)
    m_core: tokens per core (multiple of 512)
    i_core: intermediate outputs per core (multiple of 128)
    """
    if xsh_cols is None:
        xsh_cols = m_core // II
    if wsh_cols is None:
        wsh_cols = i_core // TI
    kt = h // 128          # contraction tiles
    n_it = i_core // 128   # output I-tiles (PSUM partition dim)
    n_tg = m_core // 512   # token groups (PSUM free dim)

    nc = bass.Bass(num_devices=N_CORES)
    xT = nc.dram_tensor("xT", [h, m_core], F32, kind="ExternalInput")
    wT = nc.dram_tensor("wT", [h, i_core], F32, kind="ExternalInput")
    bias = nc.dram_tensor("bias", [128, n_it], F32, kind="ExternalInput")
    outT = nc.dram_tensor("outT", [i_core, m_core], F32, kind="ExternalOutput")
    CCW = 16  # payload padded to 64 B/rank; only the first 2 floats are used
    cc_in = nc.dram_tensor("cc_in", [1, CCW], F32, kind="Internal")
    cc_out = nc.dram_tensor("cc_out", [1, CCW * N_CORES], F32, kind="Internal",
                            addr_space="Shared")
    ident = nc.inline_tensor(np.eye(128, dtype=np.float32), name="ident128")

    groups = [list(range(N_CORES))]

    with TileContext(nc) as tc:
        with (
            tc.tile_pool(name="res", bufs=1) as res,
            tc.tile_pool(name="stage", bufs=3) as stage,
            tc.tile_pool(name="small", bufs=1) as small,
            tc.tile_pool(name="psum", bufs=8, space="PSUM") as pp,
            tc.tile_pool(name="evac", bufs=4) as evac,
        ):
            # ---- phase 0: stage x fully; local maxes; AllReduce; scales ----
            # x chunks stage in SBUF during the reduction/collective so the
            # quantize pass after the scales arrive does no DMA. Core pairs
            # reduce the same token quarter (duplicated coverage is fine for a
            # max); W gets a distinct per-core 1/8 shard input instead since
            # its half streams only later during the matmul phase.
            xstage_all = res.tile([128, kt * m_core], F32, tag="xstage")
            xstage = [
                xstage_all[:, k * m_core:(k + 1) * m_core] for k in range(kt)
            ]
            macc = small.tile([128, 2 * kt], F32, tag="macc")
            # The host permutes each core's token columns so its distinct 1/8
            # max-shard sits in columns [0, xsh_cols): the AllReduce gates on
            # a quarter of the stage-in, and the rest streams during it.
            ng = kt  # staging DMA granularity: per k-tile for tight pipelining
            kg = kt // ng
            for g in range(ng):
                dst = xstage_all.rearrange(
                    "p (k c) -> p k c", k=kt
                )[:, g * kg:(g + 1) * kg, 0:xsh_cols]
                srcap = bass.AP(
                    xT, g * kg * 128 * m_core,
                    [[m_core, 128], [128 * m_core, kg], [1, xsh_cols]],
                )
                nc.sync.dma_start(dst, srcap)
                nc.vector.tensor_reduce(
                    macc[:, g * kg:(g + 1) * kg], dst,
                    axis=mybir.AxisListType.X,
                    op=mybir.AluOpType.max, apply_absolute_value=True,
                )
            # W shard = I-quad 0 of the (host-permuted) W half; its prefetch
            # doubles as the max-reduce input and is quantized after the AR
            wf0 = [
                stage.tile([128, 512], F32, tag="wf0", bufs=kt, name=f"wf0_{k}")
                for k in range(kt)
            ]
            for k in range(kt):
                nc.sync.dma_start(wf0[k][:], wT[k * 128:(k + 1) * 128, 0:512])
                nc.vector.tensor_reduce(
                    macc[:, kt + k:kt + k + 1], wf0[k][:],
                    axis=mybir.AxisListType.X,
                    op=mybir.AluOpType.max, apply_absolute_value=True,
                )
            # partition-max via PE transpose (the custom GPSIMD partition ops
            # fail codegen in this walrus build), one chain per tensor so the
            # x scalar ships to the collective input while W still reduces
            idt = small.tile([128, 128], F32, tag="idt")
            nc.sync.dma_start(idt[:], ident[:, :])
            czero = small.tile([1, CCW], F32, tag="czero", name="czero")
            nc.vector.memset(czero[:], 0.0)
            nc.sync.dma_start(cc_in[0:1, 2:CCW], czero[0:1, 2:CCW])
            gm2 = small.tile([128, 2], F32, tag="gm2")
            lmax = small.tile([1, 2], F32, tag="lmax")
            for c, (lo, hi) in enumerate([(0, kt), (kt, 2 * kt)]):
                nc.vector.tensor_reduce(
                    gm2[:, c:c + 1], macc[:, lo:hi], axis=mybir.AxisListType.X,
                    op=mybir.AluOpType.max,
                )
                gmt = pp.tile([1, 128], F32, tag="ps", name=f"gmt{c}")
                nc.tensor.transpose(gmt[:], gm2[:, c:c + 1], idt[:])
                nc.vector.tensor_reduce(
                    lmax[:, c:c + 1], gmt[:], axis=mybir.AxisListType.X,
                    op=mybir.AluOpType.max,
                )
                cc_in_dma = nc.sync.dma_start(
                    cc_in[0:1, c:c + 1], lmax[0:1, c:c + 1]
                )
            # AllGather + local max: the cost model (and HW) charges an
            # AllReduce ~1.9x an AllGather at this size
            nc.gpsimd.collective_compute(
                "AllGather", mybir.AluOpType.bypass, replica_groups=groups,
                ins=[cc_in[:, :]], outs=[cc_out[:, :]],
            )
            g6 = small.tile([128, CCW * N_CORES], F32, tag="g6")
            gmx_dma = nc.sync.dma_start(
                g6[:], cc_out[0:1, :].broadcast_to([128, CCW * N_CORES])
            )
            gmx16 = small.tile([128, CCW], F32, tag="gmx16")
            nc.vector.tensor_reduce(
                gmx16[:], g6[:, :].rearrange("p (r s) -> p s r", r=N_CORES),
                axis=mybir.AxisListType.X, op=mybir.AluOpType.max,
            )
            gmx = gmx16[:, 0:2]
            # stage the non-shard x columns during the collective window; the
            # explicit dep keeps the (serialized) DMA engines clear until the
            # tiny AllReduce input is on its way
            rw = m_core - xsh_cols
            for g in range(ng):
                dst = xstage_all.rearrange(
                    "p (k c) -> p k c", k=kt
                )[:, g * kg:(g + 1) * kg, xsh_cols:m_core]
                srcap = bass.AP(
                    xT, g * kg * 128 * m_core + xsh_cols,
                    [[m_core, 128], [128 * m_core, kg], [1, rw]],
                )
                d = nc.sync.dma_start(dst, srcap)
                gate0 = cc_in_dma if g < ng - 1 else gmx_dma
                add_dep_helper(gate0 and d.ins, gate0.ins, sync=True,
                               reason="keep DMA engines clear pre-AllReduce")
            # scales: s = gmax/127 (jnp computes max/127; mult by 1/127 is
            # within 1 ulp), inv = 1/s, ss = sx*sw
            sxsw = small.tile([128, 2], F32, tag="sxsw")
            nc.vector.tensor_scalar_mul(sxsw[:], gmx, 1.0 / 127.0)
            inv = small.tile([128, 2], F32, tag="inv")
            nc.vector.reciprocal(inv[:], sxsw[:])
            ss = small.tile([128, 1], F32, tag="ss")
            nc.vector.tensor_tensor(
                ss[:], sxsw[:, 0:1], sxsw[:, 1:2], op=mybir.AluOpType.mult
            )
            bt = small.tile([128, n_it], F32, tag="bt")
            nc.sync.dma_start(bt[:], bias[:, :])
            # ---- phase 1: quantize x from the staged chunks (no DMA) ----
            xq = res.tile([128, kt * m_core], BF16, tag="xq")  # [p, k, tok]
            wq = res.tile([128, kt * i_core], BF16, tag="wq")  # [p, k, I]
            qchunks = [(c, 512) for c in range(0, m_core, 512)]
            late_quant = []  # tg2+ chunks: defer behind W-quant (see below)
            for c0, cw in qchunks:
                for k in range(kt):
                    t1 = stage.tile([128, 512], F32, tag="t1", bufs=6)
                    a = nc.scalar.activation(
                        t1[:, :cw], xstage[k][:, c0:c0 + cw],
                        mybir.ActivationFunctionType.Copy,
                        bias=MAGIC, scale=inv[:, 0:1],
                    )
                    if c0 >= 1024:
                        late_quant.append(a)
                    nc.vector.tensor_scalar(
                        xq[:, k * m_core + c0:k * m_core + c0 + cw],
                        t1[:, :cw], MAGIC, None, op0=mybir.AluOpType.subtract,
                    )

            # ---- phase 2: stream+quantize W by I-quads; matmul + gelu ----
            # Two passes over the I-tiles: pass A covers token group 0 only
            # (gated by just 1/4 of the x-quantize), pass B the remaining
            # groups once the quantizer has finished. W streams during pass A.
            IQ = 512  # W I-columns per streamed chunk = 4 output I-tiles

            def mm_evac(i, tgs):
                ps = [
                    pp.tile([128, 512], F32, tag="ps", name=f"ps_{i}_{tg}")
                    for tg in tgs
                ]
                for k in range(kt):
                    lhsT = wq[:, k * i_core + i * 128:k * i_core + (i + 1) * 128]
                    for j, tg in enumerate(tgs):
                        rhs = xq[:, k * m_core + tg * 512:
                                 k * m_core + (tg + 1) * 512]
                        nc.tensor.matmul(
                            ps[j][:], lhsT, rhs,
                            start=(k == 0), stop=(k == kt - 1),
                        )
                for j, tg in enumerate(tgs):
                    ot = evac.tile([128, 512], F32, tag="ot")
                    nc.scalar.activation(
                        ot[:], ps[j][:],
                        mybir.ActivationFunctionType.Gelu,
                        bias=bt[:, i:i + 1], scale=ss[:, 0:1],
                    )
                    nc.sync.dma_start(
                        outT[i * 128:(i + 1) * 128, tg * 512:(tg + 1) * 512],
                        ot[:],
                    )

            for q0 in range(0, i_core, IQ):
                qw = min(IQ, i_core - q0)
                for k in range(kt):
                    if q0 == 0:
                        wf = wf0[k]
                    else:
                        wf = stage.tile([128, IQ], F32, tag="wf", bufs=8)
                        d = nc.sync.dma_start(
                            wf[:, :qw], wT[k * 128:(k + 1) * 128, q0:q0 + qw]
                        )
                        gate = gmx_dma
                        add_dep_helper(
                            d.ins, gate.ins, sync=True,
                            reason="keep DMA engines clear pre-AllReduce")
                    t2 = stage.tile([128, IQ], F32, tag="t2")
                    nc.vector.tensor_scalar(
                        t2[:, :qw], wf[:, :qw], inv[:, 1:2], MAGIC,
                        op0=mybir.AluOpType.mult, op1=mybir.AluOpType.add,
                    )
                    wlast = nc.vector.tensor_scalar(
                        wq[:, k * i_core + q0:k * i_core + q0 + qw],
                        t2[:, :qw], MAGIC, None, op0=mybir.AluOpType.subtract,
                    )
                if q0 == i_core - IQ:
                    # token groups 2-3 aren't consumed until pass B (~55us
                    # later): keep their quantize off the DVE/ACT while the
                    # W stream feeds the first matmul pass
                    for a in late_quant:
                        add_dep_helper(a.ins, wlast.ins, sync=True,
                                       reason="defer tg2+ x-quant behind W")
                for i in range(q0 // 128, (q0 + qw) // 128):
                    mm_evac(i, [0, 1])
            for tg in range(2, n_tg):
                for i in range(n_it):
                    mm_evac(i, [tg])
    _split_sync_waits(nc)
    return nc


_CACHE: dict = {}


def _get_nc():
    if "nc" not in _CACHE:
        _CACHE["nc"] = build()
    return _CACHE["nc"]


def shard_inputs(x, W, b):
    """Host-side sharding: pure layout (transpose/slice/replicate), no math."""
    x2 = np.ascontiguousarray(x.reshape(M, H).T)  # [H, M]
    in_maps = []
    for c in range(N_CORES):
        ti, ii = c // II, c % II
        mq, ih = M // TI, I // II
        q = x2[:, ti * mq:(ti + 1) * mq]
        sh = mq // II
        perm = np.r_[ii * sh:(ii + 1) * sh, 0:ii * sh, (ii + 1) * sh:mq]
        xT = np.ascontiguousarray(q[:, perm])
        # permute W columns so this core's distinct 1/8 max-shard (an I-quad)
        # is block 0: its prefetch doubles as the shard max input
        wsd = ih // TI
        wperm = np.r_[ti * wsd:(ti + 1) * wsd, 0:ti * wsd, (ti + 1) * wsd:ih]
        wT = np.ascontiguousarray(W[ii * ih:(ii + 1) * ih, :].T[:, wperm])
        bia = np.ascontiguousarray(
            b[ii * ih:(ii + 1) * ih][wperm].reshape(ih // 128, 128).T
        )
        in_maps.append({"xT": xT, "wT": wT, "bias": bia})
    return in_maps


def unshard_output(results):
    """Assemble per-core transposed blocks into the full [B, S, I] output."""
    outT = np.empty((I, M), np.float32)
    for c in range(N_CORES):
        ti, ii = c // II, c % II
        mq, ih = M // TI, I // II
        sh = mq // II
        perm = np.r_[ii * sh:(ii + 1) * sh, 0:ii * sh, (ii + 1) * sh:mq]
        wsd = ih // TI
        wperm = np.r_[ti * wsd:(ti + 1) * wsd, 0:ti * wsd, (ti + 1) * wsd:ih]
        outT[ii * ih:(ii + 1) * ih, ti * mq:(ti + 1) * mq] = \
            results[c]["outT"][np.argsort(wperm)][:, np.argsort(perm)]
    return np.ascontiguousarray(outT.T).reshape(B, S, I)


def kernel(x, W, b):
    nc = _get_nc()
    in_maps = shard_inputs(
        np.asarray(x, np.float32), np.asarray(W, np.float32), np.asarray(b, np.float32)
    )
    res = bass_utils.run_bass_kernel_spmd(nc, in_maps, core_ids=list(range(N_CORES)))
    return unshard_output(res.results)


# revision 50
# speedup vs baseline: 1.3330x; 1.0021x over previous
"""Trainium2 Bass kernel for nn_BertIntermediate (QuantizeLinear + exact GELU).

Reference computation (see harness reference):
    xq = fake_quant(x)   # symmetric per-tensor int8 fake quant, scale = max|x|/127
    Wq = fake_quant(W)
    h  = xq @ Wq.T + b
    out = h * 0.5 * (1 + erf(h/sqrt(2)))

Key numerical insight: q = round(v/scale) is an integer in [-127, 127], exactly
representable in bf16 (8-bit mantissa holds integers up to 256). Products are
<= 127*128 and k-sums over H=1024 stay below 2^24, so a bf16 matmul with fp32
PSUM accumulation reproduces the fp32 reference EXACTLY (up to rounding-tie
flips worth ~1e-3 absolute). The scales factor out of the GEMM:
    h = (sx*sw) * (qx @ qW.T) + b
and fold into the scalar-engine activation (gelu(scale*psum + bias)).

Sharding (8 cores): 2D grid, 4-way over tokens x 2-way over intermediate dim.
Per core: x^T quarter [1024, 2048] replicated x2, W^T half [1024, 2048]
replicated x4, output block written transposed [2048 I, 2048 tok].
This costs ~34 MB DMA per core vs ~53 MB for the pure Megatron column split,
balancing DMA (~100us) against PE (~110us) at the roofline ridge.

The global quantization scales need max|x|, max|W| over the FULL tensors: each
core reduces a distinct 1/8 shard (the host permutes token/I columns so every
core's shard sits in block 0 of its inputs), a PE-transpose folds the
per-partition maxes, and a tiny padded AllGather + local max across the 8
cores yields the global scales on-device.

Rounding: round-half-to-even via the fp32 magic-number trick
    rne(v) = (v + 1.5*2^23) - 1.5*2^23        (exact for |v| <= 2^22)
which matches jnp.round's banker's rounding.
"""

import numpy as np

import concourse.bass as bass
import concourse.mybir as mybir
from concourse import bass_utils
from concourse.tile import TileContext
from concourse.tile_rust import add_dep_helper

F32 = mybir.dt.float32
BF16 = mybir.dt.bfloat16
MAGIC = 12582912.0  # 1.5 * 2**23: fp32 add/sub rounds to nearest int (RNE)
N_CORES = 8
TI, II = 4, 2  # token-quarters x intermediate-halves

# Full problem dims
B, S, H, I = 16, 512, 1024, 4096
M = B * S  # 8192 tokens


def _split_sync_waits(nc, max_waits=1):
    """Walrus in this container rejects instructions carrying more than a
    couple of sync-wait commands ("Too many sync wait commands"). Hoist excess
    waits onto single-wait nops inserted just before the instruction on the
    same engine queue — sequencers process in order, so semantics are
    unchanged."""
    n = 0
    for fn in nc.m.functions:
        for blk in fn.blocks:
            new_insts = []
            for inst in blk.instructions:
                si = inst.sync_info
                waits = list(si.on_wait or []) if si is not None else []
                if len(waits) > max_waits:
                    keep = waits[-max_waits:]
                    for w in waits[:-max_waits]:
                        n += 1
                        nop = mybir.InstNoOp(
                            name=f"I-waitsplit-{n}",
                            ins=[],
                            outs=[],
                            engine=inst.engine,
                        )
                        nop.sync_info = mybir.SyncInfo(on_wait=[w], on_update=[])
                        new_insts.append(nop)
                    inst.sync_info = mybir.SyncInfo(
                        on_wait=keep, on_update=list(si.on_update or [])
                    )
                new_insts.append(inst)
            blk.instructions = new_insts


def build(h=H, m_core=M // TI, i_core=I // II, xsh_cols=None, wsh_cols=None):
    """Build the SPMD Bass program for one core's block.

    h:      contraction dim (multiple of 128)
    m_core: tokens per core (multiple of 512)
    i_core: intermediate outputs per core (multiple of 128)
    """
    if xsh_cols is None:
        xsh_cols = m_core // II
    if wsh_cols is None:
        wsh_cols = i_core // TI
    kt = h // 128          # contraction tiles
    n_it = i_core // 128   # output I-tiles (PSUM partition dim)
    n_tg = m_core // 512   # token groups (PSUM free dim)

    nc = bass.Bass(num_devices=N_CORES)
    xT = nc.dram_tensor("xT", [h, m_core], F32, kind="ExternalInput")
    wT = nc.dram_tensor("wT", [h, i_core], F32, kind="ExternalInput")
    bias = nc.dram_tensor("bias", [128, n_it], F32, kind="ExternalInput")
    outT = nc.dram_tensor("outT", [i_core, m_core], F32, kind="ExternalOutput")
    CCW = 16  # payload padded to 64 B/rank; only the first 2 floats are used
    cc_in = nc.dram_tensor("cc_in", [1, CCW], F32, kind="Internal")
    cc_out = nc.dram_tensor("cc_out", [1, CCW * N_CORES], F32, kind="Internal",
                            addr_space="Shared")
    ident = nc.inline_tensor(np.eye(128, dtype=np.float32), name="ident128")

    groups = [list(range(N_CORES))]

    with TileContext(nc) as tc:
        with (
            tc.tile_pool(name="res", bufs=1) as res,
            tc.tile_pool(name="stage", bufs=3) as stage,
            tc.tile_pool(name="small", bufs=1) as small,
            tc.tile_pool(name="psum", bufs=8, space="PSUM") as pp,
            tc.tile_pool(name="evac", bufs=4) as evac,
        ):
            # ---- phase 0: stage x fully; local maxes; AllReduce; scales ----
            # x chunks stage in SBUF during the reduction/collective so the
            # quantize pass after the scales arrive does no DMA. Core pairs
            # reduce the same token quarter (duplicated coverage is fine for a
            # max); W gets a distinct per-core 1/8 shard input instead since
            # its half streams only later during the matmul phase.
            xstage_all = res.tile([128, kt * m_core], F32, tag="xstage")
            xstage = [
                xstage_all[:, k * m_core:(k + 1) * m_core] for k in range(kt)
            ]
            macc = small.tile([128, 2 * kt], F32, tag="macc")
            # The host permutes each core's token columns so its distinct 1/8
            # max-shard sits in columns [0, xsh_cols): the AllReduce gates on
            # a quarter of the stage-in, and the rest streams during it.
            ng = kt  # staging DMA granularity: per k-tile for tight pipelining
            kg = kt // ng
            for g in range(ng):
                dst = xstage_all.rearrange(
                    "p (k c) -> p k c", k=kt
                )[:, g * kg:(g + 1) * kg, 0:xsh_cols]
                srcap = bass.AP(
                    xT, g * kg * 128 * m_core,
                    [[m_core, 128], [128 * m_core, kg], [1, xsh_cols]],
                )
                nc.sync.dma_start(dst, srcap)
                nc.vector.tensor_reduce(
                    macc[:, g * kg:(g + 1) * kg], dst,
                    axis=mybir.AxisListType.X,
                    op=mybir.AluOpType.max, apply_absolute_value=True,
                )
            # W shard = I-quad 0 of the (host-permuted) W half; its prefetch
            # doubles as the max-reduce input and is quantized after the AR
            wf0 = [
                stage.tile([128, 512], F32, tag="wf0", bufs=kt, name=f"wf0_{k}")
                for k in range(kt)
            ]
            for k in range(kt):
                nc.sync.dma_start(wf0[k][:], wT[k * 128:(k + 1) * 128, 0:512])
                nc.vector.tensor_reduce(
                    macc[:, kt + k:kt + k + 1], wf0[k][:],
                    axis=mybir.AxisListType.X,
                    op=mybir.AluOpType.max, apply_absolute_value=True,
                )
            # partition-max via PE transpose (the custom GPSIMD partition ops
            # fail codegen in this walrus build), one chain per tensor so the
            # x scalar ships to the collective input while W still reduces
            idt = small.tile([128, 128], F32, tag="idt")
            nc.sync.dma_start(idt[:], ident[:, :])
            czero = small.tile([1, CCW], F32, tag="czero", name="czero")
            nc.vector.memset(czero[:], 0.0)
            nc.sync.dma_start(cc_in[0:1, 2:CCW], czero[0:1, 2:CCW])
            gm2 = small.tile([128, 2], F32, tag="gm2")
            lmax = small.tile([1, 2], F32, tag="lmax")
            for c, (lo, hi) in enumerate([(0, kt), (kt, 2 * kt)]):
                nc.vector.tensor_reduce(
                    gm2[:, c:c + 1], macc[:, lo:hi], axis=mybir.AxisListType.X,
                    op=mybir.AluOpType.max,
                )
                gmt = pp.tile([1, 128], F32, tag="ps", name=f"gmt{c}")
                nc.tensor.transpose(gmt[:], gm2[:, c:c + 1], idt[:])
                nc.vector.tensor_reduce(
                    lmax[:, c:c + 1], gmt[:], axis=mybir.AxisListType.X,
                    op=mybir.AluOpType.max,
                )
                cc_in_dma = nc.sync.dma_start(
                    cc_in[0:1, c:c + 1], lmax[0:1, c:c + 1]
                )
            # AllGather + local max: the cost model (and HW) charges an
            # AllReduce ~1.9x an AllGather at this size
            nc.gpsimd.collective_compute(
                "AllGather", mybir.AluOpType.bypass, replica_groups=groups,
                ins=[cc_in[:, :]], outs=[cc_out[:, :]],
            )
            g6 = small.tile([128, CCW * N_CORES], F32, tag="g6")
            gmx_dma = nc.sync.dma_start(
                g6[:], cc_out[0:1, :].broadcast_to([128, CCW * N_CORES])
            )
            gmx16 = small.tile([128, CCW], F32, tag="gmx16")
            nc.vector.tensor_reduce(
                gmx16[:], g6[:, :].rearrange("p (r s) -> p s r", r=N_CORES),
                axis=mybir.AxisListType.X, op=mybir.AluOpType.max,
            )
            gmx = gmx16[:, 0:2]
            # stage the non-shard x columns during the collective window; the
            # explicit dep keeps the (serialized) DMA engines clear until the
            # tiny AllReduce input is on its way
            rw = m_core - xsh_cols
            for g in range(ng):
                dst = xstage_all.rearrange(
                    "p (k c) -> p k c", k=kt
                )[:, g * kg:(g + 1) * kg, xsh_cols:m_core]
                srcap = bass.AP(
                    xT, g * kg * 128 * m_core + xsh_cols,
                    [[m_core, 128], [128 * m_core, kg], [1, rw]],
                )
                d = nc.sync.dma_start(dst, srcap)
                gate0 = cc_in_dma if g < ng - 1 else gmx_dma
                add_dep_helper(gate0 and d.ins, gate0.ins, sync=True,
                               reason="keep DMA engines clear pre-AllReduce")
            # scales: s = gmax/127 (jnp computes max/127; mult by 1/127 is
            # within 1 ulp), inv = 1/s, ss = sx*sw
            sxsw = small.tile([128, 2], F32, tag="sxsw")
            nc.vector.tensor_scalar_mul(sxsw[:], gmx, 1.0 / 127.0)
            inv = small.tile([128, 2], F32, tag="inv")
            nc.vector.reciprocal(inv[:], sxsw[:])
            ss = small.tile([128, 1], F32, tag="ss")
            nc.vector.tensor_tensor(
                ss[:], sxsw[:, 0:1], sxsw[:, 1:2], op=mybir.AluOpType.mult
            )
            bt = small.tile([128, n_it], F32, tag="bt")
            nc.sync.dma_start(bt[:], bias[:, :])
            # ---- phase 1: quantize x from the staged chunks (no DMA) ----
            xq = res.tile([128, kt * m_core], BF16, tag="xq")  # [p, k, tok]
            wq = res.tile([128, kt * i_core], BF16, tag="wq")  # [p, k, I]
            qchunks = [(c, 512) for c in range(0, m_core, 512)]
            late_quant = []  # tg2+ chunks: defer behind W-quant (see below)
            for c0, cw in qchunks:
                for k in range(kt):
                    t1 = stage.tile([128, 512], F32, tag="t1", bufs=6)
                    a = nc.scalar.activation(
                        t1[:, :cw], xstage[k][:, c0:c0 + cw],
                        mybir.ActivationFunctionType.Copy,
                        bias=MAGIC, scale=inv[:, 0:1],
                    )
                    if c0 >= 1024:
                        late_quant.append(a)
                    nc.vector.tensor_scalar(
                        xq[:, k * m_core + c0:k * m_core + c0 + cw],
                        t1[:, :cw], MAGIC, None, op0=mybir.AluOpType.subtract,
                    )

            # ---- phase 2: stream+quantize W by I-quads; matmul + gelu ----
            # Two passes over the I-tiles: pass A covers token group 0 only
            # (gated by just 1/4 of the x-quantize), pass B the remaining
            # groups once the quantizer has finished. W streams during pass A.
            IQ = 512  # W I-columns per streamed chunk = 4 output I-tiles

            def mm_evac(i, tgs):
                ps = [
                    pp.tile([128, 512], F32, tag="ps", name=f"ps_{i}_{tg}")
                    for tg in tgs
                ]
                for k in range(kt):
                    lhsT = wq[:, k * i_core + i * 128:k * i_core + (i + 1) * 128]
                    for j, tg in enumerate(tgs):
                        rhs = xq[:, k * m_core + tg * 512:
                                 k * m_core + (tg + 1) * 512]
                        nc.tensor.matmul(
                            ps[j][:], lhsT, rhs,
                            start=(k == 0), stop=(k == kt - 1),
                        )
                for j, tg in enumerate(tgs):
                    ot = evac.tile([128, 512], F32, tag="ot")
                    nc.scalar.activation(
                        ot[:], ps[j][:],
                        mybir.ActivationFunctionType.Gelu,
                        bias=bt[:, i:i + 1], scale=ss[:, 0:1],
                    )
                    nc.sync.dma_start(
                        outT[i * 128:(i + 1) * 128, tg * 512:(tg + 1) * 512],
                        ot[:],
                    )

            for q0 in range(0, i_core, IQ):
                qw = min(IQ, i_core - q0)
                for k in range(kt):
                    if q0 == 0:
                        wf = wf0[k]
                    else:
                        wf = stage.tile([128, IQ], F32, tag="wf", bufs=8)
                        d = nc.sync.dma_start(
                            wf[:, :qw], wT[k * 128:(k + 1) * 128, q0:q0 + qw]
                        )
                        gate = gmx_dma
                        add_dep_helper(
                            d.ins, gate.ins, sync=True,
                            reason="keep DMA engines clear pre-AllReduce")
                    t2 = stage.tile([128, IQ], F32, tag="t2")
                    nc.vector.tensor_scalar(
                        t2[:, :qw], wf[:, :qw], inv[:, 1:2], MAGIC,
                        op0=mybir.AluOpType.mult, op1=mybir.AluOpType.add,
                    )
                    weng = nc.gpsimd if q0 == IQ else nc.vector
                    wlast = weng.tensor_scalar(
                        wq[:, k * i_core + q0:k * i_core + q0 + qw],
                        t2[:, :qw], MAGIC, None, op0=mybir.AluOpType.subtract,
                    )
                if q0 == i_core - IQ:
                    # token groups 2-3 aren't consumed until pass B (~55us
                    # later): keep their quantize off the DVE/ACT while the
                    # W stream feeds the first matmul pass
                    for a in late_quant:
                        add_dep_helper(a.ins, wlast.ins, sync=True,
                                       reason="defer tg2+ x-quant behind W")
                for i in range(q0 // 128, (q0 + qw) // 128):
                    mm_evac(i, [0, 1])
            for tg in range(2, n_tg):
                for i in range(n_it):
                    mm_evac(i, [tg])
    _split_sync_waits(nc)
    return nc


_CACHE: dict = {}


def _get_nc():
    if "nc" not in _CACHE:
        _CACHE["nc"] = build()
    return _CACHE["nc"]


def shard_inputs(x, W, b):
    """Host-side sharding: pure layout (transpose/slice/replicate), no math."""
    x2 = np.ascontiguousarray(x.reshape(M, H).T)  # [H, M]
    in_maps = []
    for c in range(N_CORES):
        ti, ii = c // II, c % II
        mq, ih = M // TI, I // II
        q = x2[:, ti * mq:(ti + 1) * mq]
        sh = mq // II
        perm = np.r_[ii * sh:(ii + 1) * sh, 0:ii * sh, (ii + 1) * sh:mq]
        xT = np.ascontiguousarray(q[:, perm])
        # permute W columns so this core's distinct 1/8 max-shard (an I-quad)
        # is block 0: its prefetch doubles as the shard max input
        wsd = ih // TI
        wperm = np.r_[ti * wsd:(ti + 1) * wsd, 0:ti * wsd, (ti + 1) * wsd:ih]
        wT = np.ascontiguousarray(W[ii * ih:(ii + 1) * ih, :].T[:, wperm])
        bia = np.ascontiguousarray(
            b[ii * ih:(ii + 1) * ih][wperm].reshape(ih // 128, 128).T
        )
        in_maps.append({"xT": xT, "wT": wT, "bias": bia})
    return in_maps


def unshard_output(results):
    """Assemble per-core transposed blocks into the full [B, S, I] output."""
    outT = np.empty((I, M), np.float32)
    for c in range(N_CORES):
        ti, ii = c // II, c % II
        mq, ih = M // TI, I // II
        sh = mq // II
        perm = np.r_[ii * sh:(ii + 1) * sh, 0:ii * sh, (ii + 1) * sh:mq]
        wsd = ih // TI
        wperm = np.r_[ti * wsd:(ti + 1) * wsd, 0:ti * wsd, (ti + 1) * wsd:ih]
        outT[ii * ih:(ii + 1) * ih, ti * mq:(ti + 1) * mq] = \
            results[c]["outT"][np.argsort(wperm)][:, np.argsort(perm)]
    return np.ascontiguousarray(outT.T).reshape(B, S, I)


def kernel(x, W, b):
    nc = _get_nc()
    in_maps = shard_inputs(
        np.asarray(x, np.float32), np.asarray(W, np.float32), np.asarray(b, np.float32)
    )
    res = bass_utils.run_bass_kernel_spmd(nc, in_maps, core_ids=list(range(N_CORES)))
    return unshard_output(res.results)
